# revision 1
# baseline (speedup 1.0000x reference)
"""Allegro-style GNN message passing on 8 TRN2 NeuronCores.

Strategy:
- Host: shard edges by SENDER node range (1024 nodes/core) -> sender
  segment-sums are fully core-local (no cross-core collectives).
- Within a core, group edges by 128-node sender windows; pad each
  (core, window) group to a common K_WIN with dummy edges (d=2 -> u=0 ->
  zero contribution). One-hot matmuls on TensorE do segment-sum
  (scatter) and the gather-back.
- Layer algebra: Y[:,0] == 1, so layer-1 only needs a 16-wide
  segment-sum of w1; W_lsh[1] output is dead; V1 is only needed at
  component 0 => contraction with Ytil = Y * W_lsh[0][:,0].
- Receiver scatter: node id = hi*128+lo; per edge-tile matmul with lo
  one-hot lhsT and (hi one-hot * edge_out) rhs accumulates [128,64]
  partials in PSUM; host sums the 8 per-core partials (the unshard).
- 1/sqrt(AVG_NEIGH) and the 1/sqrt(2) residual scales are folded into
  weights on the host.
"""
import math
import sys

import numpy as np

sys.path.insert(0, "/opt/trn_rl_repo")

import ml_dtypes  # noqa: E402

BF16 = ml_dtypes.bfloat16
SIM_SILU = False   # CoreSim lacks Silu; emulate with Sigmoid*z when set

N, E, MUL, H, F = 8192, 131072, 16, 256, 16
NB = 8
P6 = 6
INV = 1.0 / math.sqrt(16.0)
NC = 8
NPC = N // NC          # nodes per core
WIN = 128
NW = NPC // WIN        # windows per core
RWIN = N // WIN        # 64 receiver windows
SQ = math.sqrt(0.5)


def _host_shard(node_attrs, vectors, senders, receivers):
    """Group edges by (core, sender-window); pad to common K_WIN."""
    core = senders // NPC
    win = (senders % NPC) // WIN
    order = np.argsort(core * NW + win, kind="stable")
    key = (core * NW + win)[order]
    # group boundaries for all NC*NW groups
    counts = np.bincount(key, minlength=NC * NW)
    kwin = int(((counts.max() + 127) // 128) * 128)
    starts = np.zeros(NC * NW + 1, np.int64)
    np.cumsum(counts, out=starts[1:])

    EP = NW * kwin
    shards = []
    for c in range(NC):
        vec = np.zeros((EP, 3), np.float32)
        vec[:, 0] = 2.0
        a2 = np.zeros((EP, 2 * F), np.float32)
        sl = np.zeros(EP, np.int64)    # sender local-in-window
        rg = np.zeros(EP, np.int64)    # receiver global
        for w in range(NW):
            g = c * NW + w
            eid = order[starts[g]:starts[g + 1]]
            o = w * kwin
            n_e = len(eid)
            vec[o:o + n_e] = vectors[eid]
            a2[o:o + n_e, :F] = node_attrs[senders[eid]]
            a2[o:o + n_e, F:] = node_attrs[receivers[eid]]
            sl[o:o + n_e] = senders[eid] - (c * NPC + w * WIN)
            rg[o:o + n_e] = receivers[eid]
        shards.append((vec, a2, sl, rg))
    return kwin, shards


def _pack_core(kwin, vec, a2, sl, rg):
    """Build the per-core device arrays."""
    EP = NW * kwin
    T_ALL = EP // 128
    # plane layout: edge e = t*128 + p  ->  [3, 128, T_ALL]
    vecp = np.ascontiguousarray(
        vec.reshape(T_ALL, 128, 3).transpose(2, 1, 0)).astype(np.float32)
    attrs2 = np.ascontiguousarray(a2.T).astype(BF16)          # [32, EP]
    eye128 = np.eye(128, dtype=BF16)
    ohs = eye128[sl].reshape(T_ALL, 128, 128)                  # [t, e_p, n]
    oh_s = np.ascontiguousarray(ohs.transpose(1, 0, 2))        # [128, T, 128]
    oh_g = np.ascontiguousarray(ohs.transpose(2, 0, 1))        # [n, T, e]
    rql = eye128[rg % 128].reshape(T_ALL, 128, 128)
    rq = np.ascontiguousarray(rql.transpose(1, 0, 2))          # [128, T, 128]
    eye64 = np.eye(RWIN, dtype=BF16)
    rwl = eye64[rg // 128].reshape(T_ALL, 128, RWIN)
    rwin = np.ascontiguousarray(rwl.transpose(1, 0, 2))        # [128, T, 64]
    return dict(vecp=vecp, attrs2=attrs2, oh_s=oh_s, oh_g=oh_g,
                rq=rq, rwin=rwin)


def _prep_weights(i):
    """Fold INV and residual 1/sqrt(2) scales into weights; cast bf16."""
    w = {}
    w["we0"] = i["W_e0"].astype(BF16)                          # [40,256]
    w["we1"] = i["W_e1"].astype(BF16)                          # [256,256]
    w["wv0"] = i["W_v0"].astype(BF16)                          # [256,16]
    w["wlw0"] = (i["W_lw"][0] * INV).astype(BF16)
    w["wlw1"] = (i["W_lw"][1] * INV * SQ).astype(BF16)         # x1 = sq*x1'
    wly1_0 = i["W_ly1"][0].copy()
    wly1_1 = i["W_ly1"][1].copy()
    wly1_1[:H] *= SQ                                           # x rows scaled
    w["wly1_0"] = wly1_0.astype(BF16)
    w["wly1_1"] = wly1_1.astype(BF16)
    w["wly2_0"] = i["W_ly2"][0].astype(BF16)
    w["wly2_1"] = i["W_ly2"][1].astype(BF16)
    w["wout"] = (i["W_out"] * INV * 0.5).astype(BF16)          # x2 = .5*x2'
    w["be0"] = i["b_e0"].reshape(H, 1).astype(np.float32)
    w["be1"] = i["b_e1"].reshape(H, 1).astype(np.float32)
    w["bly1_0"] = i["b_ly1"][0].reshape(H, 1).astype(np.float32)
    w["bly1_1"] = i["b_ly1"][1].reshape(H, 1).astype(np.float32)
    w["bly2_0"] = i["b_ly2"][0].reshape(H, 1).astype(np.float32)
    w["bly2_1"] = i["b_ly2"][1].reshape(H, 1).astype(np.float32)
    w["wcol"] = np.tile(i["W_lsh"][0][:, 0].reshape(1, MUL),
                        (128, 1)).astype(np.float32)           # [128,16]
    w["ones"] = np.ones((1, 128), BF16)
    return w


_CAP_SKIP = {"InstEventSemaphore", "InstBranch", "InstNop",
             "InstCollectiveCompute"}
_CAP_LIMITS = {}


def _split_waits(nc, mybir, mk_carrier, limit=1):
    """Walrus codegen allows only 1 embedded sem-wait on compute
    instructions.  For each instruction with more, strip the extras onto
    freshly created same-engine carrier instructions inserted directly
    before it (engines are in-order, so this preserves semantics)."""
    f = nc.m.functions[0]
    made = 0
    # find blocks that carriers get appended to, to strip later
    for bb in f.blocks:
        insts = list(bb.instructions)
        plan = []          # (index, [carrier insts])
        for i, inst in enumerate(insts):
            tname = type(inst).__name__
            si = inst.sync_info
            nwait = len(si.on_wait) if (si and si.on_wait) else 0
            lim = _CAP_LIMITS.get(tname, limit)
            if tname in _CAP_SKIP or nwait <= lim:
                continue
            waits = list(si.on_wait)
            extras, keep = waits[:-lim], waits[-lim:]
            carriers = []
            for wt in extras:
                ci = mk_carrier(inst.engine)
                if ci is None:
                    keep.insert(0, wt)
                    continue
                ci.sync_info = mybir.SyncInfo(on_wait=[wt], on_update=[])
                carriers.append(ci)
                made += 1
            inst.sync_info = mybir.SyncInfo(on_wait=keep,
                                            on_update=si.on_update)
            if carriers:
                plan.append((i, carriers))
        if plan:
            new = []
            pi = 0
            pmap = dict(plan)
            for i, inst in enumerate(insts):
                if i in pmap:
                    new.extend(pmap[i])
                new.append(inst)
            bb.instructions = new
    return made


def build_graph(kwin):
    from concourse import bass, mybir
    from concourse.masks import make_identity
    from concourse.tile import TileContext

    EP = NW * kwin
    T_ALL = EP // 128
    T_W = kwin // 128
    NCH = (kwin + 511) // 512      # free chunks per window

    f32 = mybir.dt.float32
    bf16 = mybir.dt.bfloat16
    AX = mybir.AxisListType.X
    OP = mybir.AluOpType
    AF = mybir.ActivationFunctionType

    nc = bass.Bass()
    carrier_sem_cm = nc.semaphore("carrier_sem")
    carrier_sem = carrier_sem_cm.__enter__()
    dp = nc.declare_dram_parameter
    d_vecp = dp("vecp", [3, 128, T_ALL], f32, isOutput=False)
    d_attrs = dp("attrs2", [32, EP], bf16, isOutput=False)
    d_ohs = dp("oh_s", [128, T_ALL, 128], bf16, isOutput=False)
    d_ohg = dp("oh_g", [128, T_ALL, 128], bf16, isOutput=False)
    d_rq = dp("rq", [128, T_ALL, 128], bf16, isOutput=False)
    d_rwin = dp("rwin", [128, T_ALL, RWIN], bf16, isOutput=False)
    d_we0 = dp("we0", [40, H], bf16, isOutput=False)
    d_we1 = dp("we1", [H, H], bf16, isOutput=False)
    d_wv0 = dp("wv0", [H, MUL], bf16, isOutput=False)
    d_wlw0 = dp("wlw0", [H, MUL], bf16, isOutput=False)
    d_wlw1 = dp("wlw1", [H, MUL], bf16, isOutput=False)
    d_wly1 = [dp("wly1_0", [H + MUL, H], bf16, isOutput=False),
              dp("wly1_1", [H + MUL, H], bf16, isOutput=False)]
    d_wly2 = [dp("wly2_0", [H, H], bf16, isOutput=False),
              dp("wly2_1", [H, H], bf16, isOutput=False)]
    d_wout = dp("wout", [H, 1], bf16, isOutput=False)
    d_be0 = dp("be0", [H, 1], f32, isOutput=False)
    d_be1 = dp("be1", [H, 1], f32, isOutput=False)
    d_bly1 = [dp("bly1_0", [H, 1], f32, isOutput=False),
              dp("bly1_1", [H, 1], f32, isOutput=False)]
    d_bly2 = [dp("bly2_0", [H, 1], f32, isOutput=False),
              dp("bly2_1", [H, 1], f32, isOutput=False)]
    d_wcol = dp("wcol", [128, MUL], f32, isOutput=False)
    d_ones = dp("ones", [1, 128], bf16, isOutput=False)
    d_out = dp("out", [128, RWIN], f32, isOutput=True)

    with TileContext(nc) as tc:
        with (
            tc.tile_pool(name="glob", bufs=1) as gp,
            tc.tile_pool(name="wgt", bufs=1) as wp,
            tc.tile_pool(name="win", bufs=2) as wnp,
            tc.tile_pool(name="big", bufs=1) as bgp,
            tc.tile_pool(name="sml", bufs=3) as sp,
            tc.tile_pool(name="ps_mlp", bufs=2, space="PSUM") as pmlp,
            tc.tile_pool(name="ps_acc", bufs=1, space="PSUM") as pacc,
            tc.tile_pool(name="ps_gth", bufs=2, space="PSUM") as pgth,
            tc.tile_pool(name="ps_sml", bufs=2, space="PSUM") as psml,
            tc.tile_pool(name="ps_rcv", bufs=1, space="PSUM") as prcv,
        ):
            # ---------------- weights to SBUF ----------------
            def ld(d, shape, dt, tag):
                t = wp.tile(shape, dt, tag=tag)
                nc.sync.dma_start(out=t[:], in_=d[:])
                return t

            def ld2(d, cols, dt, tag, rows=H):
                # [rows, cols] -> [128, rows//128, cols] k-chunked
                nkc = rows // 128
                t = wp.tile([128, nkc, cols], dt, tag=tag)
                for kc in range(nkc):
                    nc.sync.dma_start(out=t[:, kc, :],
                                      in_=d[kc * 128:(kc + 1) * 128, :])
                return t
            we0a = ld(d_we0[0:8, :], [8, H], bf16, "we0a")
            we0b = ld(d_we0[8:40, :], [32, H], bf16, "we0b")
            we1 = ld2(d_we1, H, bf16, "we1")
            wv0 = ld2(d_wv0, MUL, bf16, "wv0")
            wlw0 = ld2(d_wlw0, MUL, bf16, "wlw0")
            wlw1 = ld2(d_wlw1, MUL, bf16, "wlw1")
            wly1 = [ld2(d_wly1[l], H, bf16, f"wly1_{l}") for l in range(2)]
            wly1fb = [ld(d_wly1[l][256:272, :], [MUL, H], bf16,
                         f"wly1fb_{l}") for l in range(2)]
            wly2 = [ld2(d_wly2[l], H, bf16, f"wly2_{l}") for l in range(2)]
            wout = ld2(d_wout, 1, bf16, "wout")
            be0 = ld2(d_be0, 1, f32, "be0")
            be1 = ld2(d_be1, 1, f32, "be1")
            bly1 = [ld2(d_bly1[l], 1, f32, f"bly1_{l}") for l in range(2)]
            bly2 = [ld2(d_bly2[l], 1, f32, f"bly2_{l}") for l in range(2)]
            wcol = ld(d_wcol, [128, MUL], f32, "wcol")
            attrs = ld(d_attrs, [32, EP], bf16, "attrs")
            ident = wp.tile([128, 128], f32, tag="ident")
            make_identity(nc, ident[:])

            ones_bf = ld(d_ones, [1, 128], bf16, "ones")

            # ---------------- edge-scalar stage (planes [128,T_ALL]) ----
            vx = gp.tile([128, T_ALL], f32)
            vy = gp.tile([128, T_ALL], f32)
            vz = gp.tile([128, T_ALL], f32)
            nc.gpsimd.dma_start(out=vx[:], in_=d_vecp[0])
            nc.gpsimd.dma_start(out=vy[:], in_=d_vecp[1])
            nc.gpsimd.dma_start(out=vz[:], in_=d_vecp[2])
            ta = gp.tile([128, T_ALL], f32)
            tb = gp.tile([128, T_ALL], f32)
            tt = nc.vector.tensor_tensor
            ts = nc.vector.tensor_scalar
            act = nc.scalar.activation

            def silu_act(out, ps_in, bias_ap):
                if not SIM_SILU:
                    act(out=out, in_=ps_in, func=AF.Silu, bias=bias_ap)
                else:
                    pp = ps_in.shape[0]
                    sg = bgp.tile([128, 512], f32, tag="simsilu")
                    zz_ = bgp.tile([128, 512], f32, tag="simsilu2")
                    cw_ = ps_in.shape[-1]
                    act(out=sg[:pp, :cw_], in_=ps_in, func=AF.Sigmoid,
                        bias=bias_ap)
                    nc.vector.tensor_scalar(out=zz_[:pp, :cw_], in0=ps_in,
                                            scalar1=bias_ap, scalar2=None,
                                            op0=OP.add)
                    nc.vector.tensor_mul(out=out, in0=sg[:pp, :cw_],
                                         in1=zz_[:pp, :cw_])
            d_pl = gp.tile([128, T_ALL], f32)
            nc.vector.tensor_mul(out=ta[:], in0=vx[:], in1=vx[:])
            nc.vector.tensor_mul(out=tb[:], in0=vy[:], in1=vy[:])
            nc.vector.tensor_add(out=ta[:], in0=ta[:], in1=tb[:])
            nc.vector.tensor_mul(out=tb[:], in0=vz[:], in1=vz[:])
            nc.vector.tensor_add(out=ta[:], in0=ta[:], in1=tb[:])
            act(out=d_pl[:], in_=ta[:], func=AF.Sqrt)
            rinv = gp.tile([128, T_ALL], f32)
            nc.vector.reciprocal(out=rinv[:], in_=d_pl[:])
            ux = gp.tile([128, T_ALL], f32)
            uy = gp.tile([128, T_ALL], f32)
            uz = gp.tile([128, T_ALL], f32)
            nc.vector.tensor_mul(out=ux[:], in0=vx[:], in1=rinv[:])
            nc.vector.tensor_mul(out=uy[:], in0=vy[:], in1=rinv[:])
            nc.vector.tensor_mul(out=uz[:], in0=vz[:], in1=rinv[:])

            # envelope u = 1 + d^6*(-28 + 48d - 21d^2), zero for d >= 1
            u_pl = gp.tile([128, T_ALL], f32)
            nc.vector.tensor_mul(out=ta[:], in0=d_pl[:], in1=d_pl[:])   # d2
            nc.vector.tensor_mul(out=tb[:], in0=ta[:], in1=d_pl[:])     # d3
            nc.vector.tensor_mul(out=tb[:], in0=tb[:], in1=tb[:])       # d6
            ts(out=ta[:], in0=ta[:], scalar1=-21.0, scalar2=None, op0=OP.mult)
            tc_q = gp.tile([128, T_ALL], f32)
            ts(out=tc_q[:], in0=d_pl[:], scalar1=48.0, scalar2=-28.0,
               op0=OP.mult, op1=OP.add)
            nc.vector.tensor_add(out=ta[:], in0=ta[:], in1=tc_q[:])
            nc.vector.tensor_mul(out=tb[:], in0=tb[:], in1=ta[:])
            ts(out=tb[:], in0=tb[:], scalar1=1.0, scalar2=None, op0=OP.add)
            ts(out=ta[:], in0=d_pl[:], scalar1=1.0, scalar2=None,
               op0=OP.is_lt)
            nc.vector.tensor_mul(out=u_pl[:], in0=tb[:], in1=ta[:])

            # spherical harmonics Y [128, T_ALL, 16] f32
            Yt = gp.tile([128, T_ALL, 16], f32)
            s3 = 3.0 ** 0.5; s5 = 5.0 ** 0.5; s15 = 15.0 ** 0.5
            s7 = 7.0 ** 0.5
            c33 = (35.0 / 8.0) ** 0.5; c32 = 105.0 ** 0.5
            c31 = (21.0 / 8.0) ** 0.5
            xx = gp.tile([128, T_ALL], f32)
            yy = gp.tile([128, T_ALL], f32)
            zz = gp.tile([128, T_ALL], f32)
            xy = gp.tile([128, T_ALL], f32)
            nc.vector.tensor_mul(out=xx[:], in0=ux[:], in1=ux[:])
            nc.vector.tensor_mul(out=yy[:], in0=uy[:], in1=uy[:])
            nc.vector.tensor_mul(out=zz[:], in0=uz[:], in1=uz[:])
            nc.vector.tensor_mul(out=xy[:], in0=ux[:], in1=uy[:])
            ts(out=Yt[:, :, 0], in0=ux[:], scalar1=0.0, scalar2=1.0,
               op0=OP.mult, op1=OP.add)
            ts(out=Yt[:, :, 1], in0=ux[:], scalar1=s3, scalar2=None,
               op0=OP.mult)
            ts(out=Yt[:, :, 2], in0=uy[:], scalar1=s3, scalar2=None,
               op0=OP.mult)
            ts(out=Yt[:, :, 3], in0=uz[:], scalar1=s3, scalar2=None,
               op0=OP.mult)
            ts(out=Yt[:, :, 4], in0=xy[:], scalar1=s15, scalar2=None,
               op0=OP.mult)
            nc.vector.tensor_mul(out=ta[:], in0=uy[:], in1=uz[:])
            ts(out=Yt[:, :, 5], in0=ta[:], scalar1=s15, scalar2=None,
               op0=OP.mult)
            ts(out=Yt[:, :, 6], in0=zz[:], scalar1=1.5 * s5,
               scalar2=-0.5 * s5, op0=OP.mult, op1=OP.add)
            nc.vector.tensor_mul(out=tb[:], in0=ux[:], in1=uz[:])
            ts(out=Yt[:, :, 7], in0=tb[:], scalar1=s15, scalar2=None,
               op0=OP.mult)
            xmy = gp.tile([128, T_ALL], f32)
            nc.vector.tensor_sub(out=xmy[:], in0=xx[:], in1=yy[:])
            ts(out=Yt[:, :, 8], in0=xmy[:], scalar1=0.5 * s15, scalar2=None,
               op0=OP.mult)
            # Y9 = c33*y*(3xx-yy)
            ts(out=ta[:], in0=xx[:], scalar1=3.0, scalar2=None, op0=OP.mult)
            nc.vector.tensor_sub(out=ta[:], in0=ta[:], in1=yy[:])
            nc.vector.tensor_mul(out=ta[:], in0=ta[:], in1=uy[:])
            ts(out=Yt[:, :, 9], in0=ta[:], scalar1=c33, scalar2=None,
               op0=OP.mult)
            # Y10 = c32*x*y*z
            nc.vector.tensor_mul(out=ta[:], in0=xy[:], in1=uz[:])
            ts(out=Yt[:, :, 10], in0=ta[:], scalar1=c32, scalar2=None,
               op0=OP.mult)
            # Y11/Y13: c31*{y,x}*(5zz-1)
            ts(out=ta[:], in0=zz[:], scalar1=5.0, scalar2=-1.0,
               op0=OP.mult, op1=OP.add)
            nc.vector.tensor_mul(out=tb[:], in0=ta[:], in1=uy[:])
            ts(out=Yt[:, :, 11], in0=tb[:], scalar1=c31, scalar2=None,
               op0=OP.mult)
            nc.vector.tensor_mul(out=tb[:], in0=ta[:], in1=ux[:])
            ts(out=Yt[:, :, 13], in0=tb[:], scalar1=c31, scalar2=None,
               op0=OP.mult)
            # Y12 = 2.5*s7*z^3 - 1.5*s7*z
            nc.vector.tensor_mul(out=ta[:], in0=zz[:], in1=uz[:])
            ts(out=ta[:], in0=ta[:], scalar1=2.5 * s7, scalar2=None,
               op0=OP.mult)
            ts(out=tb[:], in0=uz[:], scalar1=1.5 * s7, scalar2=None,
               op0=OP.mult)
            nc.vector.tensor_sub(out=Yt[:, :, 12], in0=ta[:], in1=tb[:])
            # Y14 = 0.5*c32*z*(xx-yy)
            nc.vector.tensor_mul(out=ta[:], in0=xmy[:], in1=uz[:])
            ts(out=Yt[:, :, 14], in0=ta[:], scalar1=0.5 * c32, scalar2=None,
               op0=OP.mult)
            # Y15 = c33*x*(xx-3yy)
            ts(out=ta[:], in0=yy[:], scalar1=3.0, scalar2=None, op0=OP.mult)
            nc.vector.tensor_sub(out=ta[:], in0=xx[:], in1=ta[:])
            nc.vector.tensor_mul(out=ta[:], in0=ta[:], in1=ux[:])
            ts(out=Yt[:, :, 15], in0=ta[:], scalar1=c33, scalar2=None,
               op0=OP.mult)

            # bessel (range-reduced): besu [128, T_ALL, 9]; col 8 = u
            besu = gp.tile([128, T_ALL, 8], f32)
            rs = gp.tile([128, T_ALL], f32)
            ts(out=rs[:], in0=rinv[:], scalar1=math.sqrt(2.0), scalar2=None,
               op0=OP.mult)
            mi = gp.tile([128, T_ALL], mybir.dt.int32)
            for k in range(1, NB + 1):
                ts(out=ta[:], in0=d_pl[:], scalar1=0.5 * k, scalar2=None,
                   op0=OP.mult)
                nc.vector.tensor_copy(out=mi[:], in_=ta[:])
                nc.vector.tensor_copy(out=tb[:], in_=mi[:])
                nc.vector.tensor_sub(out=ta[:], in0=ta[:], in1=tb[:])
                # ta = frac in (-0.5, 1) whether the cast rounds or truncates
                ts(out=tb[:], in0=ta[:], scalar1=0.5, scalar2=None,
                   op0=OP.is_gt)
                nc.vector.tensor_sub(out=ta[:], in0=ta[:], in1=tb[:])
                act(out=ta[:], in_=ta[:], func=AF.Sin, scale=2.0 * math.pi)
                nc.vector.tensor_mul(out=besu[:, :, k - 1], in0=ta[:],
                                      in1=rs[:])


            # ---------------- persistent receiver accumulator ----------
            ps_rcv = prcv.tile([128, RWIN], f32, space="PSUM")

            # ---------------- window loop ----------------
            for w in range(NW):
                t0 = w * T_W
                wsl = slice(w * kwin, (w + 1) * kwin)
                ohs = wnp.tile([128, T_W, 128], bf16)
                ohg = wnp.tile([128, T_W, 128], bf16)
                rqt = wnp.tile([128, T_W, 128], bf16)
                rwt = wnp.tile([128, T_W, RWIN], bf16)
                nc.sync.dma_start(out=ohs[:], in_=d_ohs[:, t0:t0 + T_W, :])
                nc.sync.dma_start(out=ohg[:], in_=d_ohg[:, t0:t0 + T_W, :])
                nc.sync.dma_start(out=rqt[:], in_=d_rq[:, t0:t0 + T_W, :])
                nc.sync.dma_start(out=rwt[:], in_=d_rwin[:, t0:t0 + T_W, :])

                # per-window feature-major bes/u rows via PE transpose
                besfm = wnp.tile([8, kwin], bf16)
                ufm = wnp.tile([1, kwin], bf16)
                for t in range(T_W):
                    pst = psml.tile([16, 128], f32, space="PSUM", tag="sml")
                    nc.tensor.transpose(out=pst[0:8, :],
                                        in_=besu[:, t0 + t, :],
                                        identity=ident[:])
                    nc.vector.tensor_copy(out=besfm[:, t * 128:(t + 1) * 128],
                                          in_=pst[0:8, :])
                    psu1 = psml.tile([16, 128], f32, space="PSUM", tag="sml")
                    nc.tensor.transpose(out=psu1[0:1, :],
                                        in_=u_pl[:, t0 + t, None],
                                        identity=ident[:])
                    nc.vector.tensor_copy(out=ufm[:, t * 128:(t + 1) * 128],
                                          in_=psu1[0:1, :])
                # broadcast u row -> [128, kwin] bf16
                ubc = bgp.tile([128, kwin], bf16)
                for ch in range(NCH):
                    c0 = ch * 512
                    c1 = min(kwin, c0 + 512)
                    psu = pmlp.tile([128, 512], f32, space="PSUM", tag="mlp")
                    nc.tensor.matmul(out=psu[:, :c1 - c0], lhsT=ones_bf[:],
                                     rhs=ufm[:, c0:c1],
                                     start=True, stop=True)
                    nc.vector.tensor_copy(out=ubc[:, c0:c1],
                                          in_=psu[:, :c1 - c0])

                # ---- edge MLP: x0 = u*silu(e1(silu(e0(bes,attrs)))) ----
                x0 = bgp.tile([128, 2, kwin], bf16)
                th = bgp.tile([128, 2, kwin], bf16)
                for ch in range(NCH):
                    c0 = ch * 512
                    c1 = min(kwin, c0 + 512)
                    cw = c1 - c0
                    for hc in range(2):
                        hs = slice(hc * 128, (hc + 1) * 128)
                        ps = pmlp.tile([128, 512], f32, space="PSUM", tag="mlp")
                        nc.tensor.matmul(out=ps[:, :cw], lhsT=we0a[:, hs],
                                         rhs=besfm[0:8, c0:c1],
                                         start=True, stop=False)
                        nc.tensor.matmul(out=ps[:, :cw], lhsT=we0b[:, hs],
                                         rhs=attrs[:, wsl][:, c0:c1],
                                         start=False, stop=True)
                        silu_act(th[:, hc, c0:c1], ps[:, :cw], be0[:, hc, :])
                for ch in range(NCH):
                    c0 = ch * 512
                    c1 = min(kwin, c0 + 512)
                    cw = c1 - c0
                    for hc in range(2):
                        hs = slice(hc * 128, (hc + 1) * 128)
                        ps = pmlp.tile([128, 512], f32, space="PSUM", tag="mlp")
                        for kc in range(2):
                            ks = slice(kc * 128, (kc + 1) * 128)
                            nc.tensor.matmul(out=ps[:, :cw],
                                             lhsT=we1[:, kc, hs],
                                             rhs=th[:, kc, c0:c1],
                                             start=(kc == 0), stop=(kc == 1))
                        silu_act(x0[:, hc, c0:c1], ps[:, :cw], be1[:, hc, :])
                for hc in range(2):
                    nc.vector.tensor_mul(out=x0[:, hc, :], in0=x0[:, hc, :],
                                          in1=ubc[:])

                # ---- xv, w0 (edge-major [128,16] per tile) ----
                xv = wnp.tile([128, T_W, MUL], f32)
                w0 = wnp.tile([128, T_W, MUL], bf16)
                for t in range(T_W):
                    tsl = slice(t * 128, (t + 1) * 128)
                    p1 = psml.tile([128, MUL], f32, space="PSUM", tag="sml")
                    p2 = psml.tile([128, MUL], f32, space="PSUM", tag="sml")
                    for kc in range(2):
                        ks = slice(kc * 128, (kc + 1) * 128)
                        nc.tensor.matmul(out=p1[:], lhsT=x0[:, kc, tsl],
                                         rhs=wv0[:, kc, :],
                                         start=(kc == 0), stop=(kc == 1))
                        nc.tensor.matmul(out=p2[:], lhsT=x0[:, kc, tsl],
                                         rhs=wlw0[:, kc, :],
                                         start=(kc == 0), stop=(kc == 1))
                    nc.vector.tensor_copy(out=xv[:, t, :], in_=p1[:])
                    nc.vector.tensor_copy(out=w0[:, t, :], in_=p2[:])

                # ---- layer-0 scatter: wY[n, m*16+i] ----
                ps_acc = pacc.tile([128, 256], f32, space="PSUM", tag="acc")
                val = wnp.tile([128, MUL, 16], bf16)
                for t in range(T_W):
                    v2 = sp.tile([128, MUL, 16], bf16)
                    nc.vector.tensor_mul(
                        out=v2[:],
                        in0=w0[:, t, :, None].to_broadcast([128, MUL, 16]),
                        in1=Yt[:, t0 + t, None, :].to_broadcast(
                            [128, MUL, 16]))
                    nc.tensor.matmul(
                        out=ps_acc[:],
                        lhsT=ohs[:, t, :],
                        rhs=v2[:].rearrange("p a b -> p (a b)"),
                        start=(t == 0), stop=(t == T_W - 1))
                wY = wnp.tile([128, 256], bf16)
                nc.vector.tensor_copy(out=wY[:], in_=ps_acc[:])

                # ---- gather + Ytil contraction + feedback ----
                V10 = wnp.tile([128, T_W, MUL], f32)
                fbfm = wnp.tile([MUL, kwin], bf16)
                prod = wnp.tile([128, MUL, 16], f32)
                ytil = wnp.tile([128, MUL], f32)
                Ssb = wnp.tile([128, MUL], f32)
                fb = wnp.tile([128, MUL], f32)
                for t in range(T_W):
                    pg = pgth.tile([128, 256], f32, space="PSUM", tag="gth")
                    nc.tensor.matmul(out=pg[:], lhsT=ohg[:, t, :], rhs=wY[:],
                                     start=True, stop=True)
                    pg3 = pg[:].rearrange("p (a b) -> p a b", b=16)
                    nc.vector.tensor_mul(out=ytil[:], in0=Yt[:, t0 + t, :],
                                          in1=wcol[:])
                    nc.vector.tensor_mul(
                        out=prod[:], in0=pg3,
                        in1=ytil[:, None, :].to_broadcast([128, MUL, 16]))
                    nc.vector.reduce_sum(out=Ssb[:, :, None], in_=prod[:],
                                         axis=AX)
                    nc.vector.tensor_mul(out=V10[:, t, :], in0=Ssb[:],
                                          in1=xv[:, t, :])
                    nc.vector.tensor_mul(out=fb[:], in0=pg3[:, :, 0],
                                          in1=xv[:, t, :])
                    pst = psml.tile([16, 128], f32, space="PSUM", tag="sml")
                    nc.tensor.transpose(out=pst[:], in_=fb[:],
                                        identity=ident[:])
                    nc.vector.tensor_copy(out=fbfm[:, t * 128:(t + 1) * 128],
                                          in_=pst[:])

                # ---- layer-0 ly1/ly2 + residual -> x1 ----
                x1 = bgp.tile([128, 2, kwin], bf16)

                def mlp_block(xin, xout, wl1, wl1fb, bl1, wl2, bl2, fbrow, resid_sq2):
                    ty = bgp.tile([128, 2, kwin], bf16)
                    for ch in range(NCH):
                        c0 = ch * 512
                        c1 = min(kwin, c0 + 512)
                        cw = c1 - c0
                        for hc in range(2):
                            hs = slice(hc * 128, (hc + 1) * 128)
                            ps = pmlp.tile([128, 512], f32, space="PSUM", tag="mlp")
                            for kc in range(2):
                                ks = slice(kc * 128, (kc + 1) * 128)
                                nc.tensor.matmul(out=ps[:, :cw],
                                                 lhsT=wl1[:, kc, hs],
                                                 rhs=xin[:, kc, c0:c1],
                                                 start=(kc == 0), stop=False)
                            nc.tensor.matmul(out=ps[:, :cw],
                                             lhsT=wl1fb[:, hs],
                                             rhs=fbrow[:, c0:c1],
                                             start=False, stop=True)
                            silu_act(ty[:, hc, c0:c1], ps[:, :cw], bl1[:, hc, :])
                    ty2 = bgp.tile([128, 2, kwin], bf16)
                    for ch in range(NCH):
                        c0 = ch * 512
                        c1 = min(kwin, c0 + 512)
                        cw = c1 - c0
                        for hc in range(2):
                            hs = slice(hc * 128, (hc + 1) * 128)
                            ps = pmlp.tile([128, 512], f32, space="PSUM", tag="mlp")
                            for kc in range(2):
                                ks = slice(kc * 128, (kc + 1) * 128)
                                nc.tensor.matmul(out=ps[:, :cw],
                                                 lhsT=wl2[:, kc, hs],
                                                 rhs=ty[:, kc, c0:c1],
                                                 start=(kc == 0),
                                                 stop=(kc == 1))
                            silu_act(ty2[:, hc, c0:c1], ps[:, :cw], bl2[:, hc, :])
                    # x_out' = x_in' + s * u * y   (s = 1 or sqrt(2))
                    for hc in range(2):
                        nc.vector.tensor_mul(out=ty2[:, hc, :],
                                              in0=ty2[:, hc, :], in1=ubc[:])
                        if resid_sq2:
                            ts(out=ty2[:, hc, :], in0=ty2[:, hc, :],
                               scalar1=math.sqrt(2.0), scalar2=None,
                               op0=OP.mult)
                        nc.vector.tensor_add(out=xout[:, hc, :],
                                             in0=xin[:, hc, :],
                                             in1=ty2[:, hc, :])

                mlp_block(x0, x1, wly1[0], wly1fb[0], bly1[0], wly2[0], bly2[0],
                          fbfm, False)

                # ---- layer 1: w1, 16-wide scatter/gather, feedback ----
                w1 = wnp.tile([128, T_W, MUL], bf16)
                for t in range(T_W):
                    tsl = slice(t * 128, (t + 1) * 128)
                    p1 = psml.tile([128, MUL], f32, space="PSUM", tag="sml")
                    for kc in range(2):
                        ks = slice(kc * 128, (kc + 1) * 128)
                        nc.tensor.matmul(out=p1[:], lhsT=x1[:, kc, tsl],
                                         rhs=wlw1[:, kc, :],
                                         start=(kc == 0), stop=(kc == 1))
                    nc.vector.tensor_copy(out=w1[:, t, :], in_=p1[:])
                ps_a1 = pacc.tile([128, 256], f32, space="PSUM", tag="acc")
                for t in range(T_W):
                    nc.tensor.matmul(out=ps_a1[:, 0:MUL], lhsT=ohs[:, t, :],
                                     rhs=w1[:, t, :],
                                     start=(t == 0), stop=(t == T_W - 1))
                wY1 = wnp.tile([128, MUL], bf16)
                nc.vector.tensor_copy(out=wY1[:], in_=ps_a1[:, 0:MUL])
                fbfm1 = wnp.tile([MUL, kwin], bf16)
                fb1 = wnp.tile([128, MUL], f32)
                for t in range(T_W):
                    pg = pgth.tile([128, 256], f32, space="PSUM", tag="gth")
                    nc.tensor.matmul(out=pg[:, 0:MUL], lhsT=ohg[:, t, :],
                                     rhs=wY1[:], start=True, stop=True)
                    nc.vector.tensor_mul(out=fb1[:], in0=pg[:, 0:MUL],
                                          in1=V10[:, t, :])
                    pst = psml.tile([16, 128], f32, space="PSUM", tag="sml")
                    nc.tensor.transpose(out=pst[:], in_=fb1[:],
                                        identity=ident[:])
                    nc.vector.tensor_copy(out=fbfm1[:, t * 128:(t + 1) * 128],
                                          in_=pst[:])

                # ---- layer-1 ly1/ly2 + residual -> x2 ----
                x2 = bgp.tile([128, 2, kwin], bf16)
                mlp_block(x1, x2, wly1[1], wly1fb[1], bly1[1], wly2[1], bly2[1],
                          fbfm1, True)

                # ---- edge out + receiver scatter ----
                eo = wnp.tile([128, 1], f32)
                mt = wnp.tile([128, RWIN], bf16)
                for t in range(T_W):
                    tsl = slice(t * 128, (t + 1) * 128)
                    p1 = psml.tile([128, MUL], f32, space="PSUM", tag="sml")
                    for kc in range(2):
                        ks = slice(kc * 128, (kc + 1) * 128)
                        nc.tensor.matmul(out=p1[:, 0:1], lhsT=x2[:, kc, tsl],
                                         rhs=wout[:, kc, :],
                                         start=(kc == 0), stop=(kc == 1))
                    nc.vector.tensor_mul(out=eo[:], in0=p1[:, 0:1],
                                          in1=u_pl[:, t0 + t, None])
                    nc.vector.tensor_mul(
                        out=mt[:], in0=rwt[:, t, :],
                        in1=eo[:].to_broadcast([128, RWIN]))
                    nc.tensor.matmul(out=ps_rcv[:], lhsT=rqt[:, t, :],
                                     rhs=mt[:],
                                     start=(w == 0 and t == 0),
                                     stop=(w == NW - 1 and t == T_W - 1))

            out_sb = gp.tile([128, RWIN], f32)
            nc.vector.tensor_copy(out=out_sb[:], in_=ps_rcv[:])
            nc.sync.dma_start(out=d_out[:], in_=out_sb[:])

    ET = mybir.EngineType
    eng_map = {ET.DVE: nc.vector, ET.Activation: nc.scalar,
               ET.Pool: nc.gpsimd, ET.PE: nc.tensor, ET.SP: nc.sync}

    def mk_carrier(eng):
        be = eng_map.get(eng)
        if be is None:
            return None
        w = be.wait_ge(carrier_sem, 0)
        ci = w.ins if hasattr(w, "ins") else w
        # strip from whatever block it was appended to
        for bb in nc.m.functions[0].blocks:
            il = list(bb.instructions)
            if any(x is ci for x in il):
                bb.instructions = [x for x in il if x is not ci]
                break
        return ci

    made = _split_waits(nc, mybir, mk_carrier)
    print(f"split_waits: carriers={made}", flush=True)
    return nc


def make_in_maps(inputs):
    kwin, shards = _host_shard(inputs["node_attrs"], inputs["vectors"],
                               inputs["senders"], inputs["receivers"])
    w = _prep_weights(inputs)
    in_maps = []
    for c in range(NC):
        m = dict(w)
        m.update(_pack_core(kwin, *shards[c]))
        in_maps.append({k: np.ascontiguousarray(v) for k, v in m.items()})
    return kwin, in_maps


def kernel(**inputs):
    inputs = {k: np.asarray(v) for k, v in inputs.items()}
    kwin, in_maps = make_in_maps(inputs)
    nc = build_graph(kwin)
    from concourse.bass_utils import run_bass_kernel_spmd
    res = run_bass_kernel_spmd(nc, in_maps, core_ids=list(range(NC)))
    out = np.zeros((128, RWIN), np.float64)
    for r in res.results:
        out += np.asarray(r["out"], np.float64)
    # node n = hi*128 + lo stored at [lo, hi]
    return np.ascontiguousarray(out.T.reshape(N, 1)).astype(np.float32)



# revision 29
# speedup vs baseline: 3.5451x; 3.5451x over previous
"""Allegro-style GNN message passing on 8 TRN2 NeuronCores.

Strategy:
- Host: shard edges by SENDER node range (1024 nodes/core) -> sender
  segment-sums are fully core-local (no cross-core collectives).
- Within a core, group edges by 128-node sender windows; pad each
  (core, window) group to a common K_WIN with dummy edges (d=2 -> u=0 ->
  zero contribution). One-hot matmuls on TensorE do segment-sum
  (scatter) and the gather-back.
- The one-hot matrices are GENERATED ON DEVICE (batched DVE is_equal
  against an iota row, DMA-xbar transpose for the gather side) from
  [128, T] index planes; sender attrs are gathered on device through
  the same one-hot matmuls. Only ~1.7 MB/core ships per call (vs ~19 MB
  with host-built one-hots).
- The axon execution path costs ~50us PER INSTRUCTION regardless of
  size, so everything batchable is batched: whole-window one-hot
  generation, 4D-broadcast products, feature-major xv/w0/w1 matmuls
  DMA-transposed back to edge-major, a single K=64 rhs (u|bessel|
  sender|receiver rows concatenated in attrs_all) for the first edge-MLP
  layer, and hoisted ytil/V10/fb products.
- Layer algebra: Y[:,0] == 1, so layer-1 only needs a 16-wide
  segment-sum of w1; W_lsh[1] output is dead; V1 is only needed at
  component 0 => contraction with Ytil = Y * W_lsh[0][:,0].
- Receiver scatter: node id = hi*128+lo; per edge-tile matmul with lo
  one-hot lhsT and (hi one-hot * edge_out) rhs accumulates [128,64]
  partials in PSUM; host sums the 8 per-core partials (the unshard).
- 1/sqrt(AVG_NEIGH) and the 1/sqrt(2) residual scales are folded into
  weights on the host.
"""
import math
import sys

import numpy as np

sys.path.insert(0, "/opt/trn_rl_repo")

import ml_dtypes  # noqa: E402

BF16 = ml_dtypes.bfloat16
SIM_SILU = False   # CoreSim lacks Silu; emulate with Sigmoid*z when set

N, E, MUL, H, F = 8192, 131072, 16, 256, 16
NB = 8
P6 = 6
INV = 1.0 / math.sqrt(16.0)
NC = 8
NPC = N // NC          # nodes per core
WIN = 128
NW = NPC // WIN        # windows per core
RWIN = N // WIN        # 64 receiver windows
SQ = math.sqrt(0.5)

# wblob row layout (all [*, H] bf16).  we0x rows (64, partition-aligned
# to match attrs_all): 0 zero (u row), 1:9 bessel, 9:32 zero,
# 32:48 sender, 48:64 receiver.
R_WE0, R_WE1 = 0, 64
R_WLY1 = [320, 592]
R_WLY2 = [864, 1120]
R_END = 1376
# fblob columns: [be0(2), be1(2), bly1_0(2), bly2_0(2), bly1_1(2),
#                 bly2_1(2), wcol(16)]
C_BE0, C_BE1 = 0, 2
C_BLY1 = [4, 8]
C_BLY2 = [6, 10]
C_WCOL = 12


def _host_shard(node_attrs, vectors, senders, receivers):
    """Group edges by (core, sender-window); pad to common K_WIN."""
    core = senders // NPC
    win = (senders % NPC) // WIN
    order = np.argsort(core * NW + win, kind="stable")
    key = (core * NW + win)[order]
    counts = np.bincount(key, minlength=NC * NW)
    kwin = int(((counts.max() + 127) // 128) * 128)
    starts = np.zeros(NC * NW + 1, np.int64)
    np.cumsum(counts, out=starts[1:])

    EP = NW * kwin
    shards = []
    for c in range(NC):
        vec = np.zeros((EP, 3), np.float32)
        vec[:, 0] = 2.0
        ra = np.zeros((EP, F), np.float32)
        sl = np.zeros(EP, np.int64)    # sender local-in-window
        rg = np.zeros(EP, np.int64)    # receiver global
        for w in range(NW):
            g = c * NW + w
            eid = order[starts[g]:starts[g + 1]]
            o = w * kwin
            n_e = len(eid)
            vec[o:o + n_e] = vectors[eid]
            ra[o:o + n_e] = node_attrs[receivers[eid]]
            sl[o:o + n_e] = senders[eid] - (c * NPC + w * WIN)
            rg[o:o + n_e] = receivers[eid]
        shards.append((vec, ra, sl, rg))
    return kwin, shards


def _pack_core(kwin, vec, ra, sl, rg):
    """Per-core device arrays: index planes + receiver attrs."""
    EP = NW * kwin
    T_ALL = EP // 128
    # plane layout: edge e = t*128 + p  ->  [128, T_ALL]
    edat = np.ascontiguousarray(
        vec.reshape(T_ALL, 128, 3).transpose(2, 1, 0)).astype(np.float32)
    eidx = np.zeros((3, 128, T_ALL), np.float32)
    eidx[0] = sl.reshape(T_ALL, 128).T
    eidx[1] = (rg % 128).reshape(T_ALL, 128).T
    eidx[2] = (rg // 128).reshape(T_ALL, 128).T
    rattr = np.ascontiguousarray(ra.T).astype(BF16)           # [16, EP]
    return dict(edat=edat, eidx=eidx.astype(BF16), rattr=rattr)


def _prep_weights(i):
    """Fold INV and residual 1/sqrt(2) scales into weights; pack blobs."""
    wb = np.zeros((R_END, H), np.float32)
    wb[1:9] = i["W_e0"][0:8]          # bessel rows; row 0 stays zero (u)
    wb[32:48] = i["W_e0"][8:24]       # sender rows
    wb[48:64] = i["W_e0"][24:40]      # receiver rows
    wb[R_WE1:R_WE1 + 256] = i["W_e1"]
    wly1_1 = i["W_ly1"][1].copy()
    wly1_1[:H] *= SQ                                          # x1 = sq*x1'
    wb[R_WLY1[0]:R_WLY1[0] + 272] = i["W_ly1"][0]
    wb[R_WLY1[1]:R_WLY1[1] + 272] = wly1_1
    wb[R_WLY2[0]:R_WLY2[0] + 256] = i["W_ly2"][0]
    wb[R_WLY2[1]:R_WLY2[1] + 256] = i["W_ly2"][1]
    ws = np.zeros((H, 49), np.float32)
    ws[:, 0:16] = i["W_v0"]
    ws[:, 16:32] = i["W_lw"][0] * INV
    ws[:, 32:48] = i["W_lw"][1] * INV * SQ
    ws[:, 48:49] = i["W_out"] * INV * 0.5                     # x2 = .5*x2'
    fb = np.zeros((128, 28), np.float32)
    for j, b in enumerate([i["b_e0"], i["b_e1"], i["b_ly1"][0],
                           i["b_ly2"][0], i["b_ly1"][1], i["b_ly2"][1]]):
        fb[:, 2 * j] = b[:128]
        fb[:, 2 * j + 1] = b[128:]
    fb[:, C_WCOL:C_WCOL + 16] = i["W_lsh"][0][:, 0][None, :]
    return dict(wblob=wb.astype(BF16), wsmall=ws.astype(BF16),
                fblob=fb.astype(np.float32))


_CAP_SKIP = {"InstEventSemaphore", "InstBranch", "InstNop",
             "InstCollectiveCompute"}
_CAP_LIMITS = {}


def _split_waits(nc, mybir, mk_carrier, limit=1):
    """Walrus codegen allows only 1 embedded sem-wait on compute
    instructions.  For each instruction with more, strip the extras onto
    freshly created same-engine carrier instructions inserted directly
    before it (engines are in-order, so this preserves semantics)."""
    f = nc.m.functions[0]
    made = 0
    # find blocks that carriers get appended to, to strip later
    for bb in f.blocks:
        insts = list(bb.instructions)
        plan = []          # (index, [carrier insts])
        for i, inst in enumerate(insts):
            tname = type(inst).__name__
            si = inst.sync_info
            nwait = len(si.on_wait) if (si and si.on_wait) else 0
            lim = _CAP_LIMITS.get(tname, limit)
            if tname in _CAP_SKIP or nwait <= lim:
                continue
            waits = list(si.on_wait)
            extras, keep = waits[:-lim], waits[-lim:]
            carriers = []
            for wt in extras:
                ci = mk_carrier(inst.engine)
                if ci is None:
                    keep.insert(0, wt)
                    continue
                ci.sync_info = mybir.SyncInfo(on_wait=[wt], on_update=[])
                carriers.append(ci)
                made += 1
            inst.sync_info = mybir.SyncInfo(on_wait=keep,
                                            on_update=si.on_update)
            if carriers:
                plan.append((i, carriers))
        if plan:
            new = []
            pmap = dict(plan)
            for i, inst in enumerate(insts):
                if i in pmap:
                    new.extend(pmap[i])
                new.append(inst)
            bb.instructions = new
    return made


def build_graph(kwin, w):
    from concourse import bass, mybir
    from concourse.masks import make_identity
    from concourse.tile import TileContext

    EP = NW * kwin
    T_ALL = EP // 128
    T_W = kwin // 128
    NCH = (kwin + 511) // 512      # free chunks per window

    f32 = mybir.dt.float32
    bf16 = mybir.dt.bfloat16
    AX = mybir.AxisListType.X
    OP = mybir.AluOpType
    AF = mybir.ActivationFunctionType

    nc = bass.Bass()
    carrier_sem_cm = nc.semaphore("carrier_sem")
    carrier_sem = carrier_sem_cm.__enter__()
    dp = nc.declare_dram_parameter
    d_edat = dp("edat", [3, 128, T_ALL], f32, isOutput=False)
    d_eidx = dp("eidx", [3, 128, T_ALL], bf16, isOutput=False)
    d_rattr = dp("rattr", [16, EP], bf16, isOutput=False)
    d_asw = dp("asw", [WIN, NW, F], bf16, isOutput=False)
    # weights ride inside the NEFF as constants -- they never transfer
    # with the per-call inputs
    d_wblob = nc.inline_tensor(np.asarray(w["wblob"]), name="wblob")
    d_wsmall = nc.inline_tensor(np.asarray(w["wsmall"]), name="wsmall")
    d_fblob = nc.inline_tensor(np.asarray(w["fblob"]), name="fblob")
    d_out = dp("out", [128, RWIN], f32, isOutput=True)

    with TileContext(nc) as tc:
        with (
            tc.tile_pool(name="glob", bufs=1) as gp,
            tc.tile_pool(name="wgt", bufs=1) as wp,
            tc.tile_pool(name="win", bufs=1) as wnp,
            tc.tile_pool(name="big", bufs=1) as bgp,
            tc.tile_pool(name="sml", bufs=3) as sp,
            tc.tile_pool(name="ps_mlp", bufs=2, space="PSUM") as pmlp,
            tc.tile_pool(name="ps_acc", bufs=1, space="PSUM") as pacc,
            tc.tile_pool(name="ps_gth", bufs=2, space="PSUM") as pgth,
            tc.tile_pool(name="ps_sml", bufs=2, space="PSUM") as psml,
            tc.tile_pool(name="ps_rcv", bufs=1, space="PSUM") as prcv,
        ):
            # ---------------- weights to SBUF ----------------
            def ldw(r0, r1, tag):
                t = wp.tile([r1 - r0, H], bf16, tag=tag)
                nc.sync.dma_start(out=t[:], in_=d_wblob[r0:r1, :])
                return t

            def ldw2(r0, tag):
                # [256, H] -> [128, 2, H] k-chunked
                t = wp.tile([128, 2, H], bf16, tag=tag)
                for kc in range(2):
                    nc.sync.dma_start(
                        out=t[:, kc, :],
                        in_=d_wblob[r0 + kc * 128:r0 + (kc + 1) * 128, :])
                return t
            we0x = ldw(0, 64, "we0x")
            we1 = ldw2(R_WE1, "we1")
            wly1 = [ldw2(R_WLY1[0], "wly1_0"), ldw2(R_WLY1[1], "wly1_1")]
            wly1fb = [ldw(R_WLY1[0] + 256, R_WLY1[0] + 272, "wly1fb_0"),
                      ldw(R_WLY1[1] + 256, R_WLY1[1] + 272, "wly1fb_1")]
            wly2 = [ldw2(R_WLY2[0], "wly2_0"), ldw2(R_WLY2[1], "wly2_1")]
            wsm = wp.tile([128, 2, 49], bf16, tag="wsm")
            for kc in range(2):
                nc.sync.dma_start(out=wsm[:, kc, :],
                                  in_=d_wsmall[kc * 128:(kc + 1) * 128, :])
            fbt = wp.tile([128, 28], f32, tag="fblob")
            nc.sync.dma_start(out=fbt[:], in_=d_fblob[:])
            asw = wp.tile([WIN, NW, F], bf16, tag="asw")
            nc.sync.dma_start(out=asw[:], in_=d_asw[:])
            # attrs_all rows (partition-aligned starts): 0 u-fm,
            # 1:9 bessel-fm, 32:48 sender attrs, 48:64 receiver attrs --
            # the single e0 rhs (K=64); rows 9:32 zeroed (zero weights)
            attrs_all = gp.tile([64, EP], bf16)
            nc.vector.memset(attrs_all[:], 0.0)
            nc.sync.dma_start(out=attrs_all[48:64, :], in_=d_rattr[:])

            bias = {
                "be0": [fbt[:, C_BE0 + h:C_BE0 + h + 1] for h in range(2)],
                "be1": [fbt[:, C_BE1 + h:C_BE1 + h + 1] for h in range(2)],
                "bly1": [[fbt[:, c + h:c + h + 1] for h in range(2)]
                         for c in C_BLY1],
                "bly2": [[fbt[:, c + h:c + h + 1] for h in range(2)]
                         for c in C_BLY2],
            }
            wcol = fbt[:, C_WCOL:C_WCOL + 16]

            ident = wp.tile([128, 128], f32, tag="ident")
            make_identity(nc, ident[:])
            ones_bf = wp.tile([1, 128], bf16, tag="ones")
            nc.vector.memset(ones_bf[:], 1.0)
            iota_f = wp.tile([128, 128], bf16, tag="iotaf")
            nc.gpsimd.iota(iota_f[:], pattern=[[1, 128]], base=0,
                           channel_multiplier=0,
                           allow_small_or_imprecise_dtypes=True)

            # ---------------- edge-scalar stage (planes [128,T_ALL]) ----
            vx = gp.tile([128, T_ALL], f32)
            vy = gp.tile([128, T_ALL], f32)
            vz = gp.tile([128, T_ALL], f32)
            nc.gpsimd.dma_start(out=vx[:], in_=d_edat[0])
            nc.gpsimd.dma_start(out=vy[:], in_=d_edat[1])
            nc.gpsimd.dma_start(out=vz[:], in_=d_edat[2])
            sl_pl = gp.tile([128, T_ALL], bf16)
            rq_pl = gp.tile([128, T_ALL], bf16)
            rw_pl = gp.tile([128, T_ALL], bf16)
            nc.gpsimd.dma_start(out=sl_pl[:], in_=d_eidx[0])
            nc.gpsimd.dma_start(out=rq_pl[:], in_=d_eidx[1])
            nc.gpsimd.dma_start(out=rw_pl[:], in_=d_eidx[2])
            ta = gp.tile([128, T_ALL], f32)
            tb = gp.tile([128, T_ALL], f32)
            tt = nc.vector.tensor_tensor
            ts = nc.vector.tensor_scalar
            act = nc.scalar.activation

            def silu_act(out, ps_in, bias_ap):
                if not SIM_SILU:
                    act(out=out, in_=ps_in, func=AF.Silu, bias=bias_ap)
                else:
                    pp = ps_in.shape[0]
                    sg = bgp.tile([128, 512], f32, tag="simsilu")
                    zz_ = bgp.tile([128, 512], f32, tag="simsilu2")
                    cw_ = ps_in.shape[-1]
                    act(out=sg[:pp, :cw_], in_=ps_in, func=AF.Sigmoid,
                        bias=bias_ap)
                    nc.vector.tensor_scalar(out=zz_[:pp, :cw_], in0=ps_in,
                                            scalar1=bias_ap, scalar2=None,
                                            op0=OP.add)
                    nc.vector.tensor_mul(out=out, in0=sg[:pp, :cw_],
                                         in1=zz_[:pp, :cw_])
            d_pl = gp.tile([128, T_ALL], f32)
            nc.vector.tensor_mul(out=ta[:], in0=vx[:], in1=vx[:])
            nc.vector.tensor_mul(out=tb[:], in0=vy[:], in1=vy[:])
            nc.vector.tensor_add(out=ta[:], in0=ta[:], in1=tb[:])
            nc.vector.tensor_mul(out=tb[:], in0=vz[:], in1=vz[:])
            nc.vector.tensor_add(out=ta[:], in0=ta[:], in1=tb[:])
            act(out=d_pl[:], in_=ta[:], func=AF.Sqrt)
            rinv = gp.tile([128, T_ALL], f32)
            nc.vector.reciprocal(out=rinv[:], in_=d_pl[:])
            ux = gp.tile([128, T_ALL], f32)
            uy = gp.tile([128, T_ALL], f32)
            uz = gp.tile([128, T_ALL], f32)
            nc.vector.tensor_mul(out=ux[:], in0=vx[:], in1=rinv[:])
            nc.vector.tensor_mul(out=uy[:], in0=vy[:], in1=rinv[:])
            nc.vector.tensor_mul(out=uz[:], in0=vz[:], in1=rinv[:])

            # besu9: col 0 envelope u, cols 1:9 bessel (transposed together)
            besu = gp.tile([128, T_ALL, 9], f32)
            # envelope u = 1 + d^6*(-28 + 48d - 21d^2), zero for d >= 1
            nc.vector.tensor_mul(out=ta[:], in0=d_pl[:], in1=d_pl[:])   # d2
            nc.vector.tensor_mul(out=tb[:], in0=ta[:], in1=d_pl[:])     # d3
            nc.vector.tensor_mul(out=tb[:], in0=tb[:], in1=tb[:])       # d6
            ts(out=ta[:], in0=ta[:], scalar1=-21.0, scalar2=None, op0=OP.mult)
            tc_q = gp.tile([128, T_ALL], f32)
            ts(out=tc_q[:], in0=d_pl[:], scalar1=48.0, scalar2=-28.0,
               op0=OP.mult, op1=OP.add)
            nc.vector.tensor_add(out=ta[:], in0=ta[:], in1=tc_q[:])
            nc.vector.tensor_mul(out=tb[:], in0=tb[:], in1=ta[:])
            ts(out=tb[:], in0=tb[:], scalar1=1.0, scalar2=None, op0=OP.add)
            ts(out=ta[:], in0=d_pl[:], scalar1=1.0, scalar2=None,
               op0=OP.is_lt)
            nc.vector.tensor_mul(out=besu[:, :, 0], in0=tb[:], in1=ta[:])

            # spherical harmonics Y [128, T_ALL, 16] f32
            Yt = gp.tile([128, T_ALL, 16], f32)
            s3 = 3.0 ** 0.5; s5 = 5.0 ** 0.5; s15 = 15.0 ** 0.5
            s7 = 7.0 ** 0.5
            c33 = (35.0 / 8.0) ** 0.5; c32 = 105.0 ** 0.5
            c31 = (21.0 / 8.0) ** 0.5
            xx = gp.tile([128, T_ALL], f32)
            yy = gp.tile([128, T_ALL], f32)
            zz = gp.tile([128, T_ALL], f32)
            xy = gp.tile([128, T_ALL], f32)
            nc.vector.tensor_mul(out=xx[:], in0=ux[:], in1=ux[:])
            nc.vector.tensor_mul(out=yy[:], in0=uy[:], in1=uy[:])
            nc.vector.tensor_mul(out=zz[:], in0=uz[:], in1=uz[:])
            nc.vector.tensor_mul(out=xy[:], in0=ux[:], in1=uy[:])
            ts(out=Yt[:, :, 0], in0=ux[:], scalar1=0.0, scalar2=1.0,
               op0=OP.mult, op1=OP.add)
            ts(out=Yt[:, :, 1], in0=ux[:], scalar1=s3, scalar2=None,
               op0=OP.mult)
            ts(out=Yt[:, :, 2], in0=uy[:], scalar1=s3, scalar2=None,
               op0=OP.mult)
            ts(out=Yt[:, :, 3], in0=uz[:], scalar1=s3, scalar2=None,
               op0=OP.mult)
            ts(out=Yt[:, :, 4], in0=xy[:], scalar1=s15, scalar2=None,
               op0=OP.mult)
            nc.vector.tensor_mul(out=ta[:], in0=uy[:], in1=uz[:])
            ts(out=Yt[:, :, 5], in0=ta[:], scalar1=s15, scalar2=None,
               op0=OP.mult)
            ts(out=Yt[:, :, 6], in0=zz[:], scalar1=1.5 * s5,
               scalar2=-0.5 * s5, op0=OP.mult, op1=OP.add)
            nc.vector.tensor_mul(out=tb[:], in0=ux[:], in1=uz[:])
            ts(out=Yt[:, :, 7], in0=tb[:], scalar1=s15, scalar2=None,
               op0=OP.mult)
            xmy = gp.tile([128, T_ALL], f32)
            nc.vector.tensor_sub(out=xmy[:], in0=xx[:], in1=yy[:])
            ts(out=Yt[:, :, 8], in0=xmy[:], scalar1=0.5 * s15, scalar2=None,
               op0=OP.mult)
            # Y9 = c33*y*(3xx-yy)
            ts(out=ta[:], in0=xx[:], scalar1=3.0, scalar2=None, op0=OP.mult)
            nc.vector.tensor_sub(out=ta[:], in0=ta[:], in1=yy[:])
            nc.vector.tensor_mul(out=ta[:], in0=ta[:], in1=uy[:])
            ts(out=Yt[:, :, 9], in0=ta[:], scalar1=c33, scalar2=None,
               op0=OP.mult)
            # Y10 = c32*x*y*z
            nc.vector.tensor_mul(out=ta[:], in0=xy[:], in1=uz[:])
            ts(out=Yt[:, :, 10], in0=ta[:], scalar1=c32, scalar2=None,
               op0=OP.mult)
            # Y11/Y13: c31*{y,x}*(5zz-1)
            ts(out=ta[:], in0=zz[:], scalar1=5.0, scalar2=-1.0,
               op0=OP.mult, op1=OP.add)
            nc.vector.tensor_mul(out=tb[:], in0=ta[:], in1=uy[:])
            ts(out=Yt[:, :, 11], in0=tb[:], scalar1=c31, scalar2=None,
               op0=OP.mult)
            nc.vector.tensor_mul(out=tb[:], in0=ta[:], in1=ux[:])
            ts(out=Yt[:, :, 13], in0=tb[:], scalar1=c31, scalar2=None,
               op0=OP.mult)
            # Y12 = 2.5*s7*z^3 - 1.5*s7*z
            nc.vector.tensor_mul(out=ta[:], in0=zz[:], in1=uz[:])
            ts(out=ta[:], in0=ta[:], scalar1=2.5 * s7, scalar2=None,
               op0=OP.mult)
            ts(out=tb[:], in0=uz[:], scalar1=1.5 * s7, scalar2=None,
               op0=OP.mult)
            nc.vector.tensor_sub(out=Yt[:, :, 12], in0=ta[:], in1=tb[:])
            # Y14 = 0.5*c32*z*(xx-yy)
            nc.vector.tensor_mul(out=ta[:], in0=xmy[:], in1=uz[:])
            ts(out=Yt[:, :, 14], in0=ta[:], scalar1=0.5 * c32, scalar2=None,
               op0=OP.mult)
            # Y15 = c33*x*(xx-3yy)
            ts(out=ta[:], in0=yy[:], scalar1=3.0, scalar2=None, op0=OP.mult)
            nc.vector.tensor_sub(out=ta[:], in0=xx[:], in1=ta[:])
            nc.vector.tensor_mul(out=ta[:], in0=ta[:], in1=ux[:])
            ts(out=Yt[:, :, 15], in0=ta[:], scalar1=c33, scalar2=None,
               op0=OP.mult)

            # bessel (range-reduced) -> besu cols 0:8
            rs = gp.tile([128, T_ALL], f32)
            ts(out=rs[:], in0=rinv[:], scalar1=math.sqrt(2.0), scalar2=None,
               op0=OP.mult)
            mi = gp.tile([128, T_ALL], mybir.dt.int32)
            for k in range(1, NB + 1):
                ts(out=ta[:], in0=d_pl[:], scalar1=0.5 * k, scalar2=None,
                   op0=OP.mult)
                nc.vector.tensor_copy(out=mi[:], in_=ta[:])
                nc.vector.tensor_copy(out=tb[:], in_=mi[:])
                nc.vector.tensor_sub(out=ta[:], in0=ta[:], in1=tb[:])
                # ta = frac in (-0.5, 1) whether the cast rounds or truncates
                ts(out=tb[:], in0=ta[:], scalar1=0.5, scalar2=None,
                   op0=OP.is_gt)
                nc.vector.tensor_sub(out=ta[:], in0=ta[:], in1=tb[:])
                act(out=ta[:], in_=ta[:], func=AF.Sin, scale=2.0 * math.pi)
                nc.vector.tensor_mul(out=besu[:, :, k], in0=ta[:],
                                      in1=rs[:])


            # ytil = Y * wcol, hoisted out of the gather loop
            ytil_g = gp.tile([128, T_ALL, 16], f32)
            nc.vector.tensor_mul(
                out=ytil_g[:], in0=Yt[:],
                in1=wcol[:, None, :].to_broadcast([128, T_ALL, 16]))

            # ---------------- persistent receiver accumulator ----------
            ps_rcv = prcv.tile([128, RWIN], f32, space="PSUM")

            # ---------------- window loop ----------------
            for w in range(NW):
                t0 = w * T_W
                wsl = slice(w * kwin, (w + 1) * kwin)
                # feature-major bes+u rows into attrs_all[0:9]
                for t in range(T_W):
                    tt_ = t0 + t
                    pst = psml.tile([16, 128], f32, space="PSUM", tag="sml")
                    nc.tensor.transpose(out=pst[0:9, :],
                                        in_=besu[:, tt_, :],
                                        identity=ident[:])
                    nc.vector.tensor_copy(
                        out=attrs_all[0:9, tt_ * 128:(tt_ + 1) * 128],
                        in_=pst[0:9, :])
                # one-hots: batched is_equal against the iota row
                ohs = wnp.tile([128, T_W, 128], bf16)
                ohg = wnp.tile([128, T_W, 128], bf16)
                rqt = wnp.tile([128, T_W, 128], bf16)
                rwt = wnp.tile([128, T_W, RWIN], bf16)
                tt(out=ohs[:],
                   in0=iota_f[:, None, :].to_broadcast([128, T_W, 128]),
                   in1=sl_pl[:, t0:t0 + T_W, None].to_broadcast(
                       [128, T_W, 128]), op=OP.is_equal)
                tt(out=rqt[:],
                   in0=iota_f[:, None, :].to_broadcast([128, T_W, 128]),
                   in1=rq_pl[:, t0:t0 + T_W, None].to_broadcast(
                       [128, T_W, 128]), op=OP.is_equal)
                tt(out=rwt[:],
                   in0=iota_f[:, None, 0:RWIN].to_broadcast(
                       [128, T_W, RWIN]),
                   in1=rw_pl[:, t0:t0 + T_W, None].to_broadcast(
                       [128, T_W, RWIN]), op=OP.is_equal)
                for t in range(T_W):
                    nc.sync.dma_start_transpose(out=ohg[:, t, :],
                                                in_=ohs[:, t, :])
                ohg_v = ohg[:].rearrange("p a b -> p (a b)")
                # sender attrs gather into attrs_all[9:25]
                for ch in range(NCH):
                    c0 = ch * 512
                    c1 = min(kwin, c0 + 512)
                    psa = pmlp.tile([128, 512], f32, space="PSUM", tag="mlp")
                    nc.tensor.matmul(out=psa[0:16, :c1 - c0],
                                     lhsT=asw[:, w, :],
                                     rhs=ohg_v[:, c0:c1],
                                     start=True, stop=True)
                    nc.vector.tensor_copy(
                        out=attrs_all[32:48, wsl][:, c0:c1],
                        in_=psa[0:16, :c1 - c0])
                # broadcast u row -> [128, kwin] bf16
                ubc = bgp.tile([128, kwin], bf16)
                for ch in range(NCH):
                    c0 = ch * 512
                    c1 = min(kwin, c0 + 512)
                    psu = pmlp.tile([128, 512], f32, space="PSUM", tag="mlp")
                    nc.tensor.matmul(out=psu[:, :c1 - c0], lhsT=ones_bf[:],
                                     rhs=attrs_all[0:1, wsl][:, c0:c1],
                                     start=True, stop=True)
                    nc.vector.tensor_copy(out=ubc[:, c0:c1],
                                          in_=psu[:, :c1 - c0])

                # ---- edge MLP: x0 = u*silu(e1(silu(e0(bes,attrs)))) ----
                x0 = bgp.tile([128, 2, kwin], bf16)
                th = bgp.tile([128, 2, kwin], bf16)
                for ch in range(NCH):
                    c0 = ch * 512
                    c1 = min(kwin, c0 + 512)
                    cw = c1 - c0
                    for hc in range(2):
                        hs = slice(hc * 128, (hc + 1) * 128)
                        ps = pmlp.tile([128, 512], f32, space="PSUM", tag="mlp")
                        nc.tensor.matmul(out=ps[:, :cw], lhsT=we0x[:, hs],
                                         rhs=attrs_all[:, wsl][:, c0:c1],
                                         start=True, stop=True)
                        silu_act(th[:, hc, c0:c1], ps[:, :cw],
                                 bias["be0"][hc])
                for ch in range(NCH):
                    c0 = ch * 512
                    c1 = min(kwin, c0 + 512)
                    cw = c1 - c0
                    for hc in range(2):
                        hs = slice(hc * 128, (hc + 1) * 128)
                        ps = pmlp.tile([128, 512], f32, space="PSUM", tag="mlp")
                        for kc in range(2):
                            nc.tensor.matmul(out=ps[:, :cw],
                                             lhsT=we1[:, kc, hs],
                                             rhs=th[:, kc, c0:c1],
                                             start=(kc == 0), stop=(kc == 1))
                        silu_act(x0[:, hc, c0:c1], ps[:, :cw],
                                 bias["be1"][hc])
                for hc in range(2):
                    nc.vector.tensor_mul(out=x0[:, hc, :], in0=x0[:, hc, :],
                                          in1=ubc[:])

                # ---- xv|w0 feature-major, DMA-transposed to edge-major ----
                xw_fm = wnp.tile([32, kwin], bf16)
                for ch in range(NCH):
                    c0 = ch * 512
                    c1 = min(kwin, c0 + 512)
                    px = pmlp.tile([128, 512], f32, space="PSUM", tag="mlp")
                    for kc in range(2):
                        nc.tensor.matmul(out=px[0:32, :c1 - c0],
                                         lhsT=wsm[:, kc, 0:32],
                                         rhs=x0[:, kc, c0:c1],
                                         start=(kc == 0), stop=(kc == 1))
                    nc.vector.tensor_copy(out=xw_fm[:, c0:c1],
                                          in_=px[0:32, :c1 - c0])
                xw = wnp.tile([128, T_W, 32], bf16)
                for t in range(T_W):
                    nc.sync.dma_start_transpose(
                        out=xw[:, t, :], in_=xw_fm[:, t * 128:(t + 1) * 128])

                # ---- layer-0 scatter: wY[n, m*16+i] ----
                v2w = wnp.tile([128, T_W, MUL, 16], bf16)
                nc.vector.tensor_mul(
                    out=v2w[:],
                    in0=xw[:, :, 16:32, None].to_broadcast(
                        [128, T_W, MUL, 16]),
                    in1=Yt[:, t0:t0 + T_W, None, :].to_broadcast(
                        [128, T_W, MUL, 16]))
                ps_acc = pacc.tile([128, 256], f32, space="PSUM", tag="acc")
                for t in range(T_W):
                    nc.tensor.matmul(
                        out=ps_acc[:],
                        lhsT=ohs[:, t, :],
                        rhs=v2w[:, t].rearrange("p a b -> p (a b)"),
                        start=(t == 0), stop=(t == T_W - 1))
                wY = wnp.tile([128, 256], bf16)
                nc.vector.tensor_copy(out=wY[:], in_=ps_acc[:])

                # ---- gather + Ytil contraction + feedback (batched) ----
                wYe = wnp.tile([128, T_W, 256], bf16)
                for t in range(T_W):
                    pg = pgth.tile([128, 256], f32, space="PSUM", tag="gth")
                    nc.tensor.matmul(out=pg[:], lhsT=ohg[:, t, :], rhs=wY[:],
                                     start=True, stop=True)
                    nc.vector.tensor_copy(out=wYe[:, t, :], in_=pg[:])
                prodw = wnp.tile([128, T_W, MUL, 16], bf16)
                nc.vector.tensor_mul(
                    out=prodw[:],
                    in0=wYe[:].rearrange("p t (a b) -> p t a b", b=16),
                    in1=ytil_g[:, t0:t0 + T_W, None, :].to_broadcast(
                        [128, T_W, MUL, 16]))
                Sw = wnp.tile([128, T_W, MUL], f32)
                nc.vector.reduce_sum(out=Sw[:, :, :, None], in_=prodw[:],
                                     axis=AX)
                V10w = wnp.tile([128, T_W, MUL], f32)
                nc.vector.tensor_mul(out=V10w[:], in0=Sw[:],
                                     in1=xw[:, :, 0:16])
                # fb feature-major directly: wYe0_fm = wY[:,0::16]^T @ ohg,
                # times xv_fm (= xw_fm rows 0:16) -- no per-tile transposes
                fbfm = wnp.tile([MUL, kwin], bf16)
                for ch in range(NCH):
                    c0 = ch * 512
                    c1 = min(kwin, c0 + 512)
                    pf = pmlp.tile([128, 512], f32, space="PSUM", tag="mlp")
                    nc.tensor.matmul(out=pf[0:MUL, :c1 - c0],
                                     lhsT=wY[:, 0:256:16],
                                     rhs=ohg_v[:, c0:c1],
                                     start=True, stop=True)
                    nc.vector.tensor_mul(out=fbfm[:, c0:c1],
                                         in0=pf[0:MUL, :c1 - c0],
                                         in1=xw_fm[0:16, c0:c1])

                # ---- layer-0 ly1/ly2 + residual -> x1 ----
                x1 = bgp.tile([128, 2, kwin], bf16)

                def mlp_block(xin, xout, wl1, wl1fb, bl1, wl2, bl2, fbrow,
                              resid_sq2):
                    ty = bgp.tile([128, 2, kwin], bf16)
                    for ch in range(NCH):
                        c0 = ch * 512
                        c1 = min(kwin, c0 + 512)
                        cw = c1 - c0
                        for hc in range(2):
                            hs = slice(hc * 128, (hc + 1) * 128)
                            ps = pmlp.tile([128, 512], f32, space="PSUM",
                                           tag="mlp")
                            for kc in range(2):
                                nc.tensor.matmul(out=ps[:, :cw],
                                                 lhsT=wl1[:, kc, hs],
                                                 rhs=xin[:, kc, c0:c1],
                                                 start=(kc == 0), stop=False)
                            nc.tensor.matmul(out=ps[:, :cw],
                                             lhsT=wl1fb[:, hs],
                                             rhs=fbrow[:, c0:c1],
                                             start=False, stop=True)
                            silu_act(ty[:, hc, c0:c1], ps[:, :cw], bl1[hc])
                    ty2 = bgp.tile([128, 2, kwin], bf16)
                    for ch in range(NCH):
                        c0 = ch * 512
                        c1 = min(kwin, c0 + 512)
                        cw = c1 - c0
                        for hc in range(2):
                            hs = slice(hc * 128, (hc + 1) * 128)
                            ps = pmlp.tile([128, 512], f32, space="PSUM",
                                           tag="mlp")
                            for kc in range(2):
                                nc.tensor.matmul(out=ps[:, :cw],
                                                 lhsT=wl2[:, kc, hs],
                                                 rhs=ty[:, kc, c0:c1],
                                                 start=(kc == 0),
                                                 stop=(kc == 1))
                            silu_act(ty2[:, hc, c0:c1], ps[:, :cw], bl2[hc])
                    # x_out' = x_in' + s * u * y   (s = 1 or sqrt(2))
                    for hc in range(2):
                        nc.vector.tensor_mul(out=ty2[:, hc, :],
                                              in0=ty2[:, hc, :], in1=ubc[:])
                        if resid_sq2:
                            ts(out=ty2[:, hc, :], in0=ty2[:, hc, :],
                               scalar1=math.sqrt(2.0), scalar2=None,
                               op0=OP.mult)
                        nc.vector.tensor_add(out=xout[:, hc, :],
                                             in0=xin[:, hc, :],
                                             in1=ty2[:, hc, :])

                mlp_block(x0, x1, wly1[0], wly1fb[0], bias["bly1"][0],
                          wly2[0], bias["bly2"][0], fbfm, False)

                # ---- layer 1: w1, 16-wide scatter/gather, feedback ----
                w1_fm = wnp.tile([MUL, kwin], bf16)
                for ch in range(NCH):
                    c0 = ch * 512
                    c1 = min(kwin, c0 + 512)
                    px = pmlp.tile([128, 512], f32, space="PSUM", tag="mlp")
                    for kc in range(2):
                        nc.tensor.matmul(out=px[0:MUL, :c1 - c0],
                                         lhsT=wsm[:, kc, 32:48],
                                         rhs=x1[:, kc, c0:c1],
                                         start=(kc == 0), stop=(kc == 1))
                    nc.vector.tensor_copy(out=w1_fm[:, c0:c1],
                                          in_=px[0:MUL, :c1 - c0])
                w1 = wnp.tile([128, T_W, MUL], bf16)
                for t in range(T_W):
                    nc.sync.dma_start_transpose(
                        out=w1[:, t, :], in_=w1_fm[:, t * 128:(t + 1) * 128])
                ps_a1 = pacc.tile([128, 256], f32, space="PSUM", tag="acc")
                for t in range(T_W):
                    nc.tensor.matmul(out=ps_a1[:, 0:MUL], lhsT=ohs[:, t, :],
                                     rhs=w1[:, t, :],
                                     start=(t == 0), stop=(t == T_W - 1))
                wY1 = wnp.tile([128, MUL], bf16)
                nc.vector.tensor_copy(out=wY1[:], in_=ps_a1[:, 0:MUL])
                w1e = wnp.tile([128, T_W, MUL], f32)
                for t in range(T_W):
                    pg = pgth.tile([128, 256], f32, space="PSUM", tag="gth")
                    nc.tensor.matmul(out=pg[:, 0:MUL], lhsT=ohg[:, t, :],
                                     rhs=wY1[:], start=True, stop=True)
                    nc.vector.tensor_copy(out=w1e[:, t, :], in_=pg[:, 0:MUL])
                fb1w = wnp.tile([128, T_W, MUL], f32)
                nc.vector.tensor_mul(out=fb1w[:], in0=w1e[:], in1=V10w[:])
                fbfm1 = wnp.tile([MUL, kwin], bf16)
                for t in range(T_W):
                    pst = psml.tile([16, 128], f32, space="PSUM", tag="sml")
                    nc.tensor.transpose(out=pst[:], in_=fb1w[:, t, :],
                                        identity=ident[:])
                    nc.vector.tensor_copy(out=fbfm1[:, t * 128:(t + 1) * 128],
                                          in_=pst[:])

                # ---- layer-1 ly1/ly2 + residual -> x2 ----
                x2 = bgp.tile([128, 2, kwin], bf16)
                mlp_block(x1, x2, wly1[1], wly1fb[1], bias["bly1"][1],
                          wly2[1], bias["bly2"][1], fbfm1, True)

                # ---- edge out feature-major (row 0 of a 16-row tile so
                # the DMA-xbar transpose is legal), u folded in place ----
                eo16 = wnp.tile([16, kwin], bf16)
                nc.vector.memset(eo16[:], 0.0)
                for ch in range(NCH):
                    c0 = ch * 512
                    c1 = min(kwin, c0 + 512)
                    pf = pmlp.tile([128, 512], f32, space="PSUM", tag="mlp")
                    for kc in range(2):
                        nc.tensor.matmul(out=pf[0:1, :c1 - c0],
                                         lhsT=wsm[:, kc, 48:49],
                                         rhs=x2[:, kc, c0:c1],
                                         start=(kc == 0), stop=(kc == 1))
                    nc.vector.tensor_mul(out=eo16[0:1, c0:c1],
                                         in0=pf[0:1, :c1 - c0],
                                         in1=attrs_all[0:1, wsl][:, c0:c1])
                eo3 = wnp.tile([128, T_W, 16], bf16)
                for t in range(T_W):
                    nc.sync.dma_start_transpose(
                        out=eo3[:, t, :], in_=eo16[:, t * 128:(t + 1) * 128])
                mtw = wnp.tile([128, T_W, RWIN], bf16)
                nc.vector.tensor_mul(
                    out=mtw[:], in0=rwt[:],
                    in1=eo3[:, :, 0, None].to_broadcast([128, T_W, RWIN]))
                for t in range(T_W):
                    nc.tensor.matmul(out=ps_rcv[:], lhsT=rqt[:, t, :],
                                     rhs=mtw[:, t, :],
                                     start=(w == 0 and t == 0),
                                     stop=(w == NW - 1 and t == T_W - 1))

            out_sb = gp.tile([128, RWIN], f32)
            nc.vector.tensor_copy(out=out_sb[:], in_=ps_rcv[:])
            nc.sync.dma_start(out=d_out[:], in_=out_sb[:])

    ET = mybir.EngineType
    eng_map = {ET.DVE: nc.vector, ET.Activation: nc.scalar,
               ET.Pool: nc.gpsimd, ET.PE: nc.tensor, ET.SP: nc.sync}

    def mk_carrier(eng):
        be = eng_map.get(eng)
        if be is None:
            return None
        w = be.wait_ge(carrier_sem, 0)
        ci = w.ins if hasattr(w, "ins") else w
        # strip from whatever block it was appended to
        for bb in nc.m.functions[0].blocks:
            il = list(bb.instructions)
            if any(x is ci for x in il):
                bb.instructions = [x for x in il if x is not ci]
                break
        return ci

    made = _split_waits(nc, mybir, mk_carrier)
    print(f"split_waits: carriers={made}", flush=True)
    return nc


def make_in_maps(inputs):
    kwin, shards = _host_shard(inputs["node_attrs"], inputs["vectors"],
                               inputs["senders"], inputs["receivers"])
    na = inputs["node_attrs"]
    in_maps = []
    for c in range(NC):
        m = _pack_core(kwin, *shards[c])
        m["asw"] = np.ascontiguousarray(
            na[c * NPC:(c + 1) * NPC].reshape(NW, WIN, F)
            .transpose(1, 0, 2)).astype(BF16)
        in_maps.append({k: np.ascontiguousarray(v) for k, v in m.items()})
    return kwin, in_maps


def kernel(**inputs):
    inputs = {k: np.asarray(v) for k, v in inputs.items()}
    kwin, in_maps = make_in_maps(inputs)
    nc = build_graph(kwin, _prep_weights(inputs))
    from concourse.bass_utils import run_bass_kernel_spmd
    res = run_bass_kernel_spmd(nc, in_maps, core_ids=list(range(NC)))
    out = np.zeros((128, RWIN), np.float64)
    for r in res.results:
        out += np.asarray(r["out"], np.float64)
    # node n = hi*128 + lo stored at [lo, hi]
    return np.ascontiguousarray(out.T.reshape(N, 1)).astype(np.float32)


# revision 30
# speedup vs baseline: 3.6241x; 1.0223x over previous
"""Allegro-style GNN message passing on 8 TRN2 NeuronCores.

Strategy:
- Host: shard edges by SENDER node range (1024 nodes/core) -> sender
  segment-sums are fully core-local (no cross-core collectives).
- Within a core, group edges by 128-node sender windows; pad each
  (core, window) group to a common K_WIN with dummy edges (d=2 -> u=0 ->
  zero contribution). One-hot matmuls on TensorE do segment-sum
  (scatter) and the gather-back.
- The one-hot matrices are GENERATED ON DEVICE (batched DVE is_equal
  against an iota row, DMA-xbar transpose for the gather side) from
  [128, T] index planes; sender attrs are gathered on device through
  the same one-hot matmuls. Only ~1.7 MB/core ships per call (vs ~19 MB
  with host-built one-hots).
- The axon execution path costs ~50us PER INSTRUCTION regardless of
  size, so everything batchable is batched: whole-window one-hot
  generation, 4D-broadcast products, feature-major xv/w0/w1 matmuls
  DMA-transposed back to edge-major, a single K=64 rhs (u|bessel|
  sender|receiver rows concatenated in attrs_all) for the first edge-MLP
  layer, and hoisted ytil/V10/fb products.
- Layer algebra: Y[:,0] == 1, so layer-1 only needs a 16-wide
  segment-sum of w1; W_lsh[1] output is dead; V1 is only needed at
  component 0 => contraction with Ytil = Y * W_lsh[0][:,0].
- Receiver scatter: node id = hi*128+lo; per edge-tile matmul with lo
  one-hot lhsT and (hi one-hot * edge_out) rhs accumulates [128,64]
  partials in PSUM; host sums the 8 per-core partials (the unshard).
- 1/sqrt(AVG_NEIGH) and the 1/sqrt(2) residual scales are folded into
  weights on the host.
"""
import math
import sys

import numpy as np

sys.path.insert(0, "/opt/trn_rl_repo")

import ml_dtypes  # noqa: E402

BF16 = ml_dtypes.bfloat16
SIM_SILU = False   # CoreSim lacks Silu; emulate with Sigmoid*z when set

N, E, MUL, H, F = 8192, 131072, 16, 256, 16
NB = 8
P6 = 6
INV = 1.0 / math.sqrt(16.0)
NC = 8
NPC = N // NC          # nodes per core
WIN = 128
NW = NPC // WIN        # windows per core
RWIN = N // WIN        # 64 receiver windows
SQ = math.sqrt(0.5)

# wblob row layout (all [*, H] bf16).  we0x rows (64, partition-aligned
# to match attrs_all): 0 zero (u row), 1:9 bessel, 9:32 zero,
# 32:48 sender, 48:64 receiver.
R_WE0, R_WE1 = 0, 64
R_WLY1 = [320, 592]
R_WLY2 = [864, 1120]
R_END = 1376
# fblob columns: [be0(2), be1(2), bly1_0(2), bly2_0(2), bly1_1(2),
#                 bly2_1(2), wcol(16)]
C_BE0, C_BE1 = 0, 2
C_BLY1 = [4, 8]
C_BLY2 = [6, 10]
C_WCOL = 12


def _host_shard(node_attrs, vectors, senders, receivers):
    """Group edges by (core, sender-window); pad to common K_WIN."""
    core = senders // NPC
    win = (senders % NPC) // WIN
    order = np.argsort(core * NW + win, kind="stable")
    key = (core * NW + win)[order]
    counts = np.bincount(key, minlength=NC * NW)
    kwin = int(((counts.max() + 127) // 128) * 128)
    starts = np.zeros(NC * NW + 1, np.int64)
    np.cumsum(counts, out=starts[1:])

    EP = NW * kwin
    shards = []
    for c in range(NC):
        vec = np.zeros((EP, 3), np.float32)
        vec[:, 0] = 2.0
        ra = np.zeros((EP, F), np.float32)
        sl = np.zeros(EP, np.int64)    # sender local-in-window
        rg = np.zeros(EP, np.int64)    # receiver global
        for w in range(NW):
            g = c * NW + w
            eid = order[starts[g]:starts[g + 1]]
            o = w * kwin
            n_e = len(eid)
            vec[o:o + n_e] = vectors[eid]
            ra[o:o + n_e] = node_attrs[receivers[eid]]
            sl[o:o + n_e] = senders[eid] - (c * NPC + w * WIN)
            rg[o:o + n_e] = receivers[eid]
        shards.append((vec, ra, sl, rg))
    return kwin, shards


def _pack_core(kwin, vec, ra, sl, rg):
    """Per-core device arrays: index planes + receiver attrs."""
    EP = NW * kwin
    T_ALL = EP // 128
    # plane layout: edge e = t*128 + p  ->  [128, T_ALL]
    edat = np.ascontiguousarray(
        vec.reshape(T_ALL, 128, 3).transpose(2, 1, 0)).astype(np.float32)
    eidx = np.zeros((3, 128, T_ALL), np.float32)
    eidx[0] = sl.reshape(T_ALL, 128).T
    eidx[1] = (rg % 128).reshape(T_ALL, 128).T
    eidx[2] = (rg // 128).reshape(T_ALL, 128).T
    rattr = np.ascontiguousarray(ra.T).astype(BF16)           # [16, EP]
    return dict(edat=edat, eidx=eidx.astype(BF16), rattr=rattr)


def _prep_weights(i):
    """Fold INV and residual 1/sqrt(2) scales into weights; pack blobs."""
    wb = np.zeros((R_END, H), np.float32)
    wb[1:9] = i["W_e0"][0:8]          # bessel rows; row 0 stays zero (u)
    wb[32:48] = i["W_e0"][8:24]       # sender rows
    wb[48:64] = i["W_e0"][24:40]      # receiver rows
    wb[R_WE1:R_WE1 + 256] = i["W_e1"]
    wly1_1 = i["W_ly1"][1].copy()
    wly1_1[:H] *= SQ                                          # x1 = sq*x1'
    wb[R_WLY1[0]:R_WLY1[0] + 272] = i["W_ly1"][0]
    wb[R_WLY1[1]:R_WLY1[1] + 272] = wly1_1
    wb[R_WLY2[0]:R_WLY2[0] + 256] = i["W_ly2"][0]
    wb[R_WLY2[1]:R_WLY2[1] + 256] = i["W_ly2"][1]
    ws = np.zeros((H, 49), np.float32)
    ws[:, 0:16] = i["W_v0"]
    ws[:, 16:32] = i["W_lw"][0] * INV
    ws[:, 32:48] = i["W_lw"][1] * INV * SQ
    ws[:, 48:49] = i["W_out"] * INV * 0.5                     # x2 = .5*x2'
    fb = np.zeros((128, 28), np.float32)
    for j, b in enumerate([i["b_e0"], i["b_e1"], i["b_ly1"][0],
                           i["b_ly2"][0], i["b_ly1"][1], i["b_ly2"][1]]):
        fb[:, 2 * j] = b[:128]
        fb[:, 2 * j + 1] = b[128:]
    fb[:, C_WCOL:C_WCOL + 16] = i["W_lsh"][0][:, 0][None, :]
    return dict(wblob=wb.astype(BF16), wsmall=ws.astype(BF16),
                fblob=fb.astype(np.float32))


_CAP_SKIP = {"InstEventSemaphore", "InstBranch", "InstNop",
             "InstCollectiveCompute"}
_CAP_LIMITS = {}


def _split_waits(nc, mybir, mk_carrier, limit=1):
    """Walrus codegen allows only 1 embedded sem-wait on compute
    instructions.  For each instruction with more, strip the extras onto
    freshly created same-engine carrier instructions inserted directly
    before it (engines are in-order, so this preserves semantics)."""
    f = nc.m.functions[0]
    made = 0
    # find blocks that carriers get appended to, to strip later
    for bb in f.blocks:
        insts = list(bb.instructions)
        plan = []          # (index, [carrier insts])
        for i, inst in enumerate(insts):
            tname = type(inst).__name__
            si = inst.sync_info
            nwait = len(si.on_wait) if (si and si.on_wait) else 0
            lim = _CAP_LIMITS.get(tname, limit)
            if tname in _CAP_SKIP or nwait <= lim:
                continue
            waits = list(si.on_wait)
            extras, keep = waits[:-lim], waits[-lim:]
            carriers = []
            for wt in extras:
                ci = mk_carrier(inst.engine)
                if ci is None:
                    keep.insert(0, wt)
                    continue
                ci.sync_info = mybir.SyncInfo(on_wait=[wt], on_update=[])
                carriers.append(ci)
                made += 1
            inst.sync_info = mybir.SyncInfo(on_wait=keep,
                                            on_update=si.on_update)
            if carriers:
                plan.append((i, carriers))
        if plan:
            new = []
            pmap = dict(plan)
            for i, inst in enumerate(insts):
                if i in pmap:
                    new.extend(pmap[i])
                new.append(inst)
            bb.instructions = new
    return made


def build_graph(kwin, w):
    from concourse import bass, mybir
    from concourse.masks import make_identity
    from concourse.tile import TileContext

    EP = NW * kwin
    T_ALL = EP // 128
    T_W = kwin // 128
    NCH = (kwin + 511) // 512      # free chunks per window

    f32 = mybir.dt.float32
    bf16 = mybir.dt.bfloat16
    AX = mybir.AxisListType.X
    OP = mybir.AluOpType
    AF = mybir.ActivationFunctionType

    nc = bass.Bass()
    carrier_sem_cm = nc.semaphore("carrier_sem")
    carrier_sem = carrier_sem_cm.__enter__()
    dp = nc.declare_dram_parameter
    d_edat = dp("edat", [3, 128, T_ALL], f32, isOutput=False)
    d_eidx = dp("eidx", [3, 128, T_ALL], bf16, isOutput=False)
    d_rattr = dp("rattr", [16, EP], bf16, isOutput=False)
    d_asw = dp("asw", [WIN, NW, F], bf16, isOutput=False)
    # weights ride inside the NEFF as constants -- they never transfer
    # with the per-call inputs
    d_wblob = nc.inline_tensor(np.asarray(w["wblob"]), name="wblob")
    d_wsmall = nc.inline_tensor(np.asarray(w["wsmall"]), name="wsmall")
    d_fblob = nc.inline_tensor(np.asarray(w["fblob"]), name="fblob")
    d_out = dp("out", [128, RWIN], f32, isOutput=True)

    with TileContext(nc) as tc:
        with (
            tc.tile_pool(name="glob", bufs=1) as gp,
            tc.tile_pool(name="wgt", bufs=1) as wp,
            tc.tile_pool(name="win", bufs=1) as wnp,
            tc.tile_pool(name="big", bufs=1) as bgp,
            tc.tile_pool(name="sml", bufs=3) as sp,
            tc.tile_pool(name="ps_mlp", bufs=2, space="PSUM") as pmlp,
            tc.tile_pool(name="ps_acc", bufs=1, space="PSUM") as pacc,
            tc.tile_pool(name="ps_gth", bufs=2, space="PSUM") as pgth,
            tc.tile_pool(name="ps_sml", bufs=2, space="PSUM") as psml,
            tc.tile_pool(name="ps_rcv", bufs=1, space="PSUM") as prcv,
        ):
            # ---------------- weights to SBUF ----------------
            def ldw(r0, r1, tag):
                t = wp.tile([r1 - r0, H], bf16, tag=tag)
                nc.sync.dma_start(out=t[:], in_=d_wblob[r0:r1, :])
                return t

            def ldw2(r0, tag):
                # [256, H] -> [128, 2, H] k-chunked
                t = wp.tile([128, 2, H], bf16, tag=tag)
                for kc in range(2):
                    nc.sync.dma_start(
                        out=t[:, kc, :],
                        in_=d_wblob[r0 + kc * 128:r0 + (kc + 1) * 128, :])
                return t
            we0x = ldw(0, 64, "we0x")
            we1 = ldw2(R_WE1, "we1")
            wly1 = [ldw2(R_WLY1[0], "wly1_0"), ldw2(R_WLY1[1], "wly1_1")]
            wly1fb = [ldw(R_WLY1[0] + 256, R_WLY1[0] + 272, "wly1fb_0"),
                      ldw(R_WLY1[1] + 256, R_WLY1[1] + 272, "wly1fb_1")]
            wly2 = [ldw2(R_WLY2[0], "wly2_0"), ldw2(R_WLY2[1], "wly2_1")]
            wsm = wp.tile([128, 2, 49], bf16, tag="wsm")
            for kc in range(2):
                nc.sync.dma_start(out=wsm[:, kc, :],
                                  in_=d_wsmall[kc * 128:(kc + 1) * 128, :])
            fbt = wp.tile([128, 28], f32, tag="fblob")
            nc.sync.dma_start(out=fbt[:], in_=d_fblob[:])
            asw = wp.tile([WIN, NW, F], bf16, tag="asw")
            nc.sync.dma_start(out=asw[:], in_=d_asw[:])
            # attrs_all rows (partition-aligned starts): 0 u-fm,
            # 1:9 bessel-fm, 32:48 sender attrs, 48:64 receiver attrs --
            # the single e0 rhs (K=64); rows 9:32 zeroed (zero weights)
            attrs_all = gp.tile([64, EP], bf16)
            nc.vector.memset(attrs_all[:], 0.0)
            nc.sync.dma_start(out=attrs_all[48:64, :], in_=d_rattr[:])

            bias = {
                "be0": [fbt[:, C_BE0 + h:C_BE0 + h + 1] for h in range(2)],
                "be1": [fbt[:, C_BE1 + h:C_BE1 + h + 1] for h in range(2)],
                "bly1": [[fbt[:, c + h:c + h + 1] for h in range(2)]
                         for c in C_BLY1],
                "bly2": [[fbt[:, c + h:c + h + 1] for h in range(2)]
                         for c in C_BLY2],
            }
            wcol = fbt[:, C_WCOL:C_WCOL + 16]

            ident = wp.tile([128, 128], f32, tag="ident")
            make_identity(nc, ident[:])
            ones_bf = wp.tile([1, 128], bf16, tag="ones")
            nc.vector.memset(ones_bf[:], 1.0)
            iota_f = wp.tile([128, 128], bf16, tag="iotaf")
            nc.gpsimd.iota(iota_f[:], pattern=[[1, 128]], base=0,
                           channel_multiplier=0,
                           allow_small_or_imprecise_dtypes=True)

            # ---------------- edge-scalar stage (planes [128,T_ALL]) ----
            vx = gp.tile([128, T_ALL], f32)
            vy = gp.tile([128, T_ALL], f32)
            vz = gp.tile([128, T_ALL], f32)
            nc.gpsimd.dma_start(out=vx[:], in_=d_edat[0])
            nc.gpsimd.dma_start(out=vy[:], in_=d_edat[1])
            nc.gpsimd.dma_start(out=vz[:], in_=d_edat[2])
            sl_pl = gp.tile([128, T_ALL], bf16)
            rq_pl = gp.tile([128, T_ALL], bf16)
            rw_pl = gp.tile([128, T_ALL], bf16)
            nc.gpsimd.dma_start(out=sl_pl[:], in_=d_eidx[0])
            nc.gpsimd.dma_start(out=rq_pl[:], in_=d_eidx[1])
            nc.gpsimd.dma_start(out=rw_pl[:], in_=d_eidx[2])
            ta = gp.tile([128, T_ALL], f32)
            tb = gp.tile([128, T_ALL], f32)
            tt = nc.vector.tensor_tensor
            ts = nc.vector.tensor_scalar
            act = nc.scalar.activation

            def silu_act(out, ps_in, bias_ap):
                if not SIM_SILU:
                    act(out=out, in_=ps_in, func=AF.Silu, bias=bias_ap)
                else:
                    pp = ps_in.shape[0]
                    sg = bgp.tile([128, 512], f32, tag="simsilu")
                    zz_ = bgp.tile([128, 512], f32, tag="simsilu2")
                    cw_ = ps_in.shape[-1]
                    act(out=sg[:pp, :cw_], in_=ps_in, func=AF.Sigmoid,
                        bias=bias_ap)
                    nc.vector.tensor_scalar(out=zz_[:pp, :cw_], in0=ps_in,
                                            scalar1=bias_ap, scalar2=None,
                                            op0=OP.add)
                    nc.vector.tensor_mul(out=out, in0=sg[:pp, :cw_],
                                         in1=zz_[:pp, :cw_])
            d_pl = gp.tile([128, T_ALL], f32)
            nc.vector.tensor_mul(out=ta[:], in0=vx[:], in1=vx[:])
            nc.vector.tensor_mul(out=tb[:], in0=vy[:], in1=vy[:])
            nc.vector.tensor_add(out=ta[:], in0=ta[:], in1=tb[:])
            nc.vector.tensor_mul(out=tb[:], in0=vz[:], in1=vz[:])
            nc.vector.tensor_add(out=ta[:], in0=ta[:], in1=tb[:])
            act(out=d_pl[:], in_=ta[:], func=AF.Sqrt)
            rinv = gp.tile([128, T_ALL], f32)
            nc.vector.reciprocal(out=rinv[:], in_=d_pl[:])
            ux = gp.tile([128, T_ALL], f32)
            uy = gp.tile([128, T_ALL], f32)
            uz = gp.tile([128, T_ALL], f32)
            nc.vector.tensor_mul(out=ux[:], in0=vx[:], in1=rinv[:])
            nc.vector.tensor_mul(out=uy[:], in0=vy[:], in1=rinv[:])
            nc.vector.tensor_mul(out=uz[:], in0=vz[:], in1=rinv[:])

            # besu9: col 0 envelope u, cols 1:9 bessel (transposed together)
            besu = gp.tile([128, T_ALL, 9], f32)
            # envelope u = 1 + d^6*(-28 + 48d - 21d^2), zero for d >= 1
            nc.vector.tensor_mul(out=ta[:], in0=d_pl[:], in1=d_pl[:])   # d2
            nc.vector.tensor_mul(out=tb[:], in0=ta[:], in1=d_pl[:])     # d3
            nc.vector.tensor_mul(out=tb[:], in0=tb[:], in1=tb[:])       # d6
            ts(out=ta[:], in0=ta[:], scalar1=-21.0, scalar2=None, op0=OP.mult)
            tc_q = gp.tile([128, T_ALL], f32)
            ts(out=tc_q[:], in0=d_pl[:], scalar1=48.0, scalar2=-28.0,
               op0=OP.mult, op1=OP.add)
            nc.vector.tensor_add(out=ta[:], in0=ta[:], in1=tc_q[:])
            nc.vector.tensor_mul(out=tb[:], in0=tb[:], in1=ta[:])
            ts(out=tb[:], in0=tb[:], scalar1=1.0, scalar2=None, op0=OP.add)
            ts(out=ta[:], in0=d_pl[:], scalar1=1.0, scalar2=None,
               op0=OP.is_lt)
            nc.vector.tensor_mul(out=besu[:, :, 0], in0=tb[:], in1=ta[:])

            # spherical harmonics Y [128, T_ALL, 16] f32
            Yt = gp.tile([128, T_ALL, 16], f32)
            s3 = 3.0 ** 0.5; s5 = 5.0 ** 0.5; s15 = 15.0 ** 0.5
            s7 = 7.0 ** 0.5
            c33 = (35.0 / 8.0) ** 0.5; c32 = 105.0 ** 0.5
            c31 = (21.0 / 8.0) ** 0.5
            xx = gp.tile([128, T_ALL], f32)
            yy = gp.tile([128, T_ALL], f32)
            zz = gp.tile([128, T_ALL], f32)
            xy = gp.tile([128, T_ALL], f32)
            nc.vector.tensor_mul(out=xx[:], in0=ux[:], in1=ux[:])
            nc.vector.tensor_mul(out=yy[:], in0=uy[:], in1=uy[:])
            nc.vector.tensor_mul(out=zz[:], in0=uz[:], in1=uz[:])
            nc.vector.tensor_mul(out=xy[:], in0=ux[:], in1=uy[:])
            ts(out=Yt[:, :, 0], in0=ux[:], scalar1=0.0, scalar2=1.0,
               op0=OP.mult, op1=OP.add)
            ts(out=Yt[:, :, 1], in0=ux[:], scalar1=s3, scalar2=None,
               op0=OP.mult)
            ts(out=Yt[:, :, 2], in0=uy[:], scalar1=s3, scalar2=None,
               op0=OP.mult)
            ts(out=Yt[:, :, 3], in0=uz[:], scalar1=s3, scalar2=None,
               op0=OP.mult)
            ts(out=Yt[:, :, 4], in0=xy[:], scalar1=s15, scalar2=None,
               op0=OP.mult)
            nc.vector.tensor_mul(out=ta[:], in0=uy[:], in1=uz[:])
            ts(out=Yt[:, :, 5], in0=ta[:], scalar1=s15, scalar2=None,
               op0=OP.mult)
            ts(out=Yt[:, :, 6], in0=zz[:], scalar1=1.5 * s5,
               scalar2=-0.5 * s5, op0=OP.mult, op1=OP.add)
            nc.vector.tensor_mul(out=tb[:], in0=ux[:], in1=uz[:])
            ts(out=Yt[:, :, 7], in0=tb[:], scalar1=s15, scalar2=None,
               op0=OP.mult)
            xmy = gp.tile([128, T_ALL], f32)
            nc.vector.tensor_sub(out=xmy[:], in0=xx[:], in1=yy[:])
            ts(out=Yt[:, :, 8], in0=xmy[:], scalar1=0.5 * s15, scalar2=None,
               op0=OP.mult)
            # Y9 = c33*y*(3xx-yy)
            ts(out=ta[:], in0=xx[:], scalar1=3.0, scalar2=None, op0=OP.mult)
            nc.vector.tensor_sub(out=ta[:], in0=ta[:], in1=yy[:])
            nc.vector.tensor_mul(out=ta[:], in0=ta[:], in1=uy[:])
            ts(out=Yt[:, :, 9], in0=ta[:], scalar1=c33, scalar2=None,
               op0=OP.mult)
            # Y10 = c32*x*y*z
            nc.vector.tensor_mul(out=ta[:], in0=xy[:], in1=uz[:])
            ts(out=Yt[:, :, 10], in0=ta[:], scalar1=c32, scalar2=None,
               op0=OP.mult)
            # Y11/Y13: c31*{y,x}*(5zz-1)
            ts(out=ta[:], in0=zz[:], scalar1=5.0, scalar2=-1.0,
               op0=OP.mult, op1=OP.add)
            nc.vector.tensor_mul(out=tb[:], in0=ta[:], in1=uy[:])
            ts(out=Yt[:, :, 11], in0=tb[:], scalar1=c31, scalar2=None,
               op0=OP.mult)
            nc.vector.tensor_mul(out=tb[:], in0=ta[:], in1=ux[:])
            ts(out=Yt[:, :, 13], in0=tb[:], scalar1=c31, scalar2=None,
               op0=OP.mult)
            # Y12 = 2.5*s7*z^3 - 1.5*s7*z
            nc.vector.tensor_mul(out=ta[:], in0=zz[:], in1=uz[:])
            ts(out=ta[:], in0=ta[:], scalar1=2.5 * s7, scalar2=None,
               op0=OP.mult)
            ts(out=tb[:], in0=uz[:], scalar1=1.5 * s7, scalar2=None,
               op0=OP.mult)
            nc.vector.tensor_sub(out=Yt[:, :, 12], in0=ta[:], in1=tb[:])
            # Y14 = 0.5*c32*z*(xx-yy)
            nc.vector.tensor_mul(out=ta[:], in0=xmy[:], in1=uz[:])
            ts(out=Yt[:, :, 14], in0=ta[:], scalar1=0.5 * c32, scalar2=None,
               op0=OP.mult)
            # Y15 = c33*x*(xx-3yy)
            ts(out=ta[:], in0=yy[:], scalar1=3.0, scalar2=None, op0=OP.mult)
            nc.vector.tensor_sub(out=ta[:], in0=xx[:], in1=ta[:])
            nc.vector.tensor_mul(out=ta[:], in0=ta[:], in1=ux[:])
            ts(out=Yt[:, :, 15], in0=ta[:], scalar1=c33, scalar2=None,
               op0=OP.mult)

            # bessel (range-reduced) -> besu cols 0:8
            rs = gp.tile([128, T_ALL], f32)
            ts(out=rs[:], in0=rinv[:], scalar1=math.sqrt(2.0), scalar2=None,
               op0=OP.mult)
            mi = gp.tile([128, T_ALL], mybir.dt.int32)
            for k in range(1, NB + 1):
                ts(out=ta[:], in0=d_pl[:], scalar1=0.5 * k, scalar2=None,
                   op0=OP.mult)
                nc.vector.tensor_copy(out=mi[:], in_=ta[:])
                nc.vector.tensor_copy(out=tb[:], in_=mi[:])
                nc.vector.tensor_sub(out=ta[:], in0=ta[:], in1=tb[:])
                # ta = frac in (-0.5, 1) whether the cast rounds or truncates
                ts(out=tb[:], in0=ta[:], scalar1=0.5, scalar2=None,
                   op0=OP.is_gt)
                nc.vector.tensor_sub(out=ta[:], in0=ta[:], in1=tb[:])
                act(out=ta[:], in_=ta[:], func=AF.Sin, scale=2.0 * math.pi)
                nc.vector.tensor_mul(out=besu[:, :, k], in0=ta[:],
                                      in1=rs[:])


            # ytil = Y * wcol, hoisted out of the gather loop
            ytil_g = gp.tile([128, T_ALL, 16], f32)
            nc.vector.tensor_mul(
                out=ytil_g[:], in0=Yt[:],
                in1=wcol[:, None, :].to_broadcast([128, T_ALL, 16]))

            # ---------------- persistent receiver accumulator ----------
            ps_rcv = prcv.tile([128, RWIN], f32, space="PSUM")

            # ---------------- window loop ----------------
            for w in range(NW):
                t0 = w * T_W
                wsl = slice(w * kwin, (w + 1) * kwin)
                # feature-major bes+u rows into attrs_all[0:9]
                for t in range(T_W):
                    tt_ = t0 + t
                    pst = psml.tile([16, 128], f32, space="PSUM", tag="sml")
                    nc.tensor.transpose(out=pst[0:9, :],
                                        in_=besu[:, tt_, :],
                                        identity=ident[:])
                    nc.vector.tensor_copy(
                        out=attrs_all[0:9, tt_ * 128:(tt_ + 1) * 128],
                        in_=pst[0:9, :])
                # one-hots: batched is_equal against the iota row
                ohs = wnp.tile([128, T_W, 128], bf16)
                ohg = wnp.tile([128, T_W, 128], bf16)
                rqt = wnp.tile([128, T_W, 128], bf16)
                rwt = wnp.tile([128, T_W, RWIN], bf16)
                tt(out=ohs[:],
                   in0=iota_f[:, None, :].to_broadcast([128, T_W, 128]),
                   in1=sl_pl[:, t0:t0 + T_W, None].to_broadcast(
                       [128, T_W, 128]), op=OP.is_equal)
                tt(out=rqt[:],
                   in0=iota_f[:, None, :].to_broadcast([128, T_W, 128]),
                   in1=rq_pl[:, t0:t0 + T_W, None].to_broadcast(
                       [128, T_W, 128]), op=OP.is_equal)
                tt(out=rwt[:],
                   in0=iota_f[:, None, 0:RWIN].to_broadcast(
                       [128, T_W, RWIN]),
                   in1=rw_pl[:, t0:t0 + T_W, None].to_broadcast(
                       [128, T_W, RWIN]), op=OP.is_equal)
                for t in range(T_W):
                    nc.sync.dma_start_transpose(out=ohg[:, t, :],
                                                in_=ohs[:, t, :])
                ohg_v = ohg[:].rearrange("p a b -> p (a b)")
                # sender attrs gather into attrs_all[9:25]
                for ch in range(NCH):
                    c0 = ch * 512
                    c1 = min(kwin, c0 + 512)
                    psa = pmlp.tile([128, 512], f32, space="PSUM", tag="mlp")
                    nc.tensor.matmul(out=psa[0:16, :c1 - c0],
                                     lhsT=asw[:, w, :],
                                     rhs=ohg_v[:, c0:c1],
                                     start=True, stop=True)
                    nc.vector.tensor_copy(
                        out=attrs_all[32:48, wsl][:, c0:c1],
                        in_=psa[0:16, :c1 - c0])
                # broadcast u row -> [128, kwin] bf16
                ubc = bgp.tile([128, kwin], bf16)
                for ch in range(NCH):
                    c0 = ch * 512
                    c1 = min(kwin, c0 + 512)
                    psu = pmlp.tile([128, 512], f32, space="PSUM", tag="mlp")
                    nc.tensor.matmul(out=psu[:, :c1 - c0], lhsT=ones_bf[:],
                                     rhs=attrs_all[0:1, wsl][:, c0:c1],
                                     start=True, stop=True)
                    nc.vector.tensor_copy(out=ubc[:, c0:c1],
                                          in_=psu[:, :c1 - c0])

                # ---- edge MLP: x0 = u*silu(e1(silu(e0(bes,attrs)))) ----
                x0 = bgp.tile([128, 2, kwin], bf16)
                th = bgp.tile([128, 2, kwin], bf16)
                for ch in range(NCH):
                    c0 = ch * 512
                    c1 = min(kwin, c0 + 512)
                    cw = c1 - c0
                    for hc in range(2):
                        hs = slice(hc * 128, (hc + 1) * 128)
                        ps = pmlp.tile([128, 512], f32, space="PSUM", tag="mlp")
                        nc.tensor.matmul(out=ps[:, :cw], lhsT=we0x[:, hs],
                                         rhs=attrs_all[:, wsl][:, c0:c1],
                                         start=True, stop=True)
                        silu_act(th[:, hc, c0:c1], ps[:, :cw],
                                 bias["be0"][hc])
                for ch in range(NCH):
                    c0 = ch * 512
                    c1 = min(kwin, c0 + 512)
                    cw = c1 - c0
                    for hc in range(2):
                        hs = slice(hc * 128, (hc + 1) * 128)
                        ps = pmlp.tile([128, 512], f32, space="PSUM", tag="mlp")
                        for kc in range(2):
                            nc.tensor.matmul(out=ps[:, :cw],
                                             lhsT=we1[:, kc, hs],
                                             rhs=th[:, kc, c0:c1],
                                             start=(kc == 0), stop=(kc == 1))
                        silu_act(x0[:, hc, c0:c1], ps[:, :cw],
                                 bias["be1"][hc])
                nc.vector.tensor_mul(
                    out=x0[:], in0=x0[:],
                    in1=ubc[:, None, :].to_broadcast([128, 2, kwin]))

                # ---- xv|w0 feature-major, DMA-transposed to edge-major ----
                xw_fm = wnp.tile([32, kwin], bf16)
                for ch in range(NCH):
                    c0 = ch * 512
                    c1 = min(kwin, c0 + 512)
                    px = pmlp.tile([128, 512], f32, space="PSUM", tag="mlp")
                    for kc in range(2):
                        nc.tensor.matmul(out=px[0:32, :c1 - c0],
                                         lhsT=wsm[:, kc, 0:32],
                                         rhs=x0[:, kc, c0:c1],
                                         start=(kc == 0), stop=(kc == 1))
                    nc.vector.tensor_copy(out=xw_fm[:, c0:c1],
                                          in_=px[0:32, :c1 - c0])
                xw = wnp.tile([128, T_W, 32], bf16)
                for t in range(T_W):
                    nc.sync.dma_start_transpose(
                        out=xw[:, t, :], in_=xw_fm[:, t * 128:(t + 1) * 128])

                # ---- layer-0 scatter: wY[n, m*16+i] ----
                v2w = wnp.tile([128, T_W, MUL, 16], bf16)
                nc.vector.tensor_mul(
                    out=v2w[:],
                    in0=xw[:, :, 16:32, None].to_broadcast(
                        [128, T_W, MUL, 16]),
                    in1=Yt[:, t0:t0 + T_W, None, :].to_broadcast(
                        [128, T_W, MUL, 16]))
                ps_acc = pacc.tile([128, 256], f32, space="PSUM", tag="acc")
                for t in range(T_W):
                    nc.tensor.matmul(
                        out=ps_acc[:],
                        lhsT=ohs[:, t, :],
                        rhs=v2w[:, t].rearrange("p a b -> p (a b)"),
                        start=(t == 0), stop=(t == T_W - 1))
                wY = wnp.tile([128, 256], bf16)
                nc.vector.tensor_copy(out=wY[:], in_=ps_acc[:])

                # ---- gather + Ytil contraction + feedback (batched) ----
                wYe = wnp.tile([128, T_W, 256], bf16)
                for t in range(T_W):
                    pg = pgth.tile([128, 256], f32, space="PSUM", tag="gth")
                    nc.tensor.matmul(out=pg[:], lhsT=ohg[:, t, :], rhs=wY[:],
                                     start=True, stop=True)
                    nc.vector.tensor_copy(out=wYe[:, t, :], in_=pg[:])
                prodw = wnp.tile([128, T_W, MUL, 16], bf16)
                nc.vector.tensor_mul(
                    out=prodw[:],
                    in0=wYe[:].rearrange("p t (a b) -> p t a b", b=16),
                    in1=ytil_g[:, t0:t0 + T_W, None, :].to_broadcast(
                        [128, T_W, MUL, 16]))
                Sw = wnp.tile([128, T_W, MUL], f32)
                nc.vector.reduce_sum(out=Sw[:, :, :, None], in_=prodw[:],
                                     axis=AX)
                V10w = wnp.tile([128, T_W, MUL], f32)
                nc.vector.tensor_mul(out=V10w[:], in0=Sw[:],
                                     in1=xw[:, :, 0:16])
                # fb feature-major directly: wYe0_fm = wY[:,0::16]^T @ ohg,
                # times xv_fm (= xw_fm rows 0:16) -- no per-tile transposes
                fbfm = wnp.tile([MUL, kwin], bf16)
                for ch in range(NCH):
                    c0 = ch * 512
                    c1 = min(kwin, c0 + 512)
                    pf = pmlp.tile([128, 512], f32, space="PSUM", tag="mlp")
                    nc.tensor.matmul(out=pf[0:MUL, :c1 - c0],
                                     lhsT=wY[:, 0:256:16],
                                     rhs=ohg_v[:, c0:c1],
                                     start=True, stop=True)
                    nc.vector.tensor_mul(out=fbfm[:, c0:c1],
                                         in0=pf[0:MUL, :c1 - c0],
                                         in1=xw_fm[0:16, c0:c1])

                # ---- layer-0 ly1/ly2 + residual -> x1 ----
                x1 = bgp.tile([128, 2, kwin], bf16)

                def mlp_block(xin, xout, wl1, wl1fb, bl1, wl2, bl2, fbrow,
                              resid_sq2):
                    ty = bgp.tile([128, 2, kwin], bf16)
                    for ch in range(NCH):
                        c0 = ch * 512
                        c1 = min(kwin, c0 + 512)
                        cw = c1 - c0
                        for hc in range(2):
                            hs = slice(hc * 128, (hc + 1) * 128)
                            ps = pmlp.tile([128, 512], f32, space="PSUM",
                                           tag="mlp")
                            for kc in range(2):
                                nc.tensor.matmul(out=ps[:, :cw],
                                                 lhsT=wl1[:, kc, hs],
                                                 rhs=xin[:, kc, c0:c1],
                                                 start=(kc == 0), stop=False)
                            nc.tensor.matmul(out=ps[:, :cw],
                                             lhsT=wl1fb[:, hs],
                                             rhs=fbrow[:, c0:c1],
                                             start=False, stop=True)
                            silu_act(ty[:, hc, c0:c1], ps[:, :cw], bl1[hc])
                    ty2 = bgp.tile([128, 2, kwin], bf16)
                    for ch in range(NCH):
                        c0 = ch * 512
                        c1 = min(kwin, c0 + 512)
                        cw = c1 - c0
                        for hc in range(2):
                            hs = slice(hc * 128, (hc + 1) * 128)
                            ps = pmlp.tile([128, 512], f32, space="PSUM",
                                           tag="mlp")
                            for kc in range(2):
                                nc.tensor.matmul(out=ps[:, :cw],
                                                 lhsT=wl2[:, kc, hs],
                                                 rhs=ty[:, kc, c0:c1],
                                                 start=(kc == 0),
                                                 stop=(kc == 1))
                            silu_act(ty2[:, hc, c0:c1], ps[:, :cw], bl2[hc])
                    # x_out' = x_in' + s * u * y   (s = 1 or sqrt(2))
                    nc.vector.tensor_mul(
                        out=ty2[:], in0=ty2[:],
                        in1=ubc[:, None, :].to_broadcast([128, 2, kwin]))
                    if resid_sq2:
                        ts(out=ty2[:], in0=ty2[:], scalar1=math.sqrt(2.0),
                           scalar2=None, op0=OP.mult)
                    nc.vector.tensor_add(out=xout[:], in0=xin[:],
                                         in1=ty2[:])

                mlp_block(x0, x1, wly1[0], wly1fb[0], bias["bly1"][0],
                          wly2[0], bias["bly2"][0], fbfm, False)

                # ---- layer 1: w1, 16-wide scatter/gather, feedback ----
                w1_fm = wnp.tile([MUL, kwin], bf16)
                for ch in range(NCH):
                    c0 = ch * 512
                    c1 = min(kwin, c0 + 512)
                    px = pmlp.tile([128, 512], f32, space="PSUM", tag="mlp")
                    for kc in range(2):
                        nc.tensor.matmul(out=px[0:MUL, :c1 - c0],
                                         lhsT=wsm[:, kc, 32:48],
                                         rhs=x1[:, kc, c0:c1],
                                         start=(kc == 0), stop=(kc == 1))
                    nc.vector.tensor_copy(out=w1_fm[:, c0:c1],
                                          in_=px[0:MUL, :c1 - c0])
                w1 = wnp.tile([128, T_W, MUL], bf16)
                for t in range(T_W):
                    nc.sync.dma_start_transpose(
                        out=w1[:, t, :], in_=w1_fm[:, t * 128:(t + 1) * 128])
                ps_a1 = pacc.tile([128, 256], f32, space="PSUM", tag="acc")
                for t in range(T_W):
                    nc.tensor.matmul(out=ps_a1[:, 0:MUL], lhsT=ohs[:, t, :],
                                     rhs=w1[:, t, :],
                                     start=(t == 0), stop=(t == T_W - 1))
                wY1 = wnp.tile([128, MUL], bf16)
                nc.vector.tensor_copy(out=wY1[:], in_=ps_a1[:, 0:MUL])
                w1e = wnp.tile([128, T_W, MUL], f32)
                for t in range(T_W):
                    pg = pgth.tile([128, 256], f32, space="PSUM", tag="gth")
                    nc.tensor.matmul(out=pg[:, 0:MUL], lhsT=ohg[:, t, :],
                                     rhs=wY1[:], start=True, stop=True)
                    nc.vector.tensor_copy(out=w1e[:, t, :], in_=pg[:, 0:MUL])
                fb1w = wnp.tile([128, T_W, MUL], f32)
                nc.vector.tensor_mul(out=fb1w[:], in0=w1e[:], in1=V10w[:])
                fbfm1 = wnp.tile([MUL, kwin], bf16)
                for t in range(T_W):
                    pst = psml.tile([16, 128], f32, space="PSUM", tag="sml")
                    nc.tensor.transpose(out=pst[:], in_=fb1w[:, t, :],
                                        identity=ident[:])
                    nc.vector.tensor_copy(out=fbfm1[:, t * 128:(t + 1) * 128],
                                          in_=pst[:])

                # ---- layer-1 ly1/ly2 + residual -> x2 ----
                x2 = bgp.tile([128, 2, kwin], bf16)
                mlp_block(x1, x2, wly1[1], wly1fb[1], bias["bly1"][1],
                          wly2[1], bias["bly2"][1], fbfm1, True)

                # ---- edge out feature-major (row 0 of a 16-row tile so
                # the DMA-xbar transpose is legal), u folded in place ----
                eo16 = wnp.tile([16, kwin], bf16)
                nc.vector.memset(eo16[:], 0.0)
                for ch in range(NCH):
                    c0 = ch * 512
                    c1 = min(kwin, c0 + 512)
                    pf = pmlp.tile([128, 512], f32, space="PSUM", tag="mlp")
                    for kc in range(2):
                        nc.tensor.matmul(out=pf[0:1, :c1 - c0],
                                         lhsT=wsm[:, kc, 48:49],
                                         rhs=x2[:, kc, c0:c1],
                                         start=(kc == 0), stop=(kc == 1))
                    nc.vector.tensor_mul(out=eo16[0:1, c0:c1],
                                         in0=pf[0:1, :c1 - c0],
                                         in1=attrs_all[0:1, wsl][:, c0:c1])
                eo3 = wnp.tile([128, T_W, 16], bf16)
                for t in range(T_W):
                    nc.sync.dma_start_transpose(
                        out=eo3[:, t, :], in_=eo16[:, t * 128:(t + 1) * 128])
                mtw = wnp.tile([128, T_W, RWIN], bf16)
                nc.vector.tensor_mul(
                    out=mtw[:], in0=rwt[:],
                    in1=eo3[:, :, 0, None].to_broadcast([128, T_W, RWIN]))
                for t in range(T_W):
                    nc.tensor.matmul(out=ps_rcv[:], lhsT=rqt[:, t, :],
                                     rhs=mtw[:, t, :],
                                     start=(w == 0 and t == 0),
                                     stop=(w == NW - 1 and t == T_W - 1))

            out_sb = gp.tile([128, RWIN], f32)
            nc.vector.tensor_copy(out=out_sb[:], in_=ps_rcv[:])
            nc.sync.dma_start(out=d_out[:], in_=out_sb[:])

    ET = mybir.EngineType
    eng_map = {ET.DVE: nc.vector, ET.Activation: nc.scalar,
               ET.Pool: nc.gpsimd, ET.PE: nc.tensor, ET.SP: nc.sync}

    def mk_carrier(eng):
        be = eng_map.get(eng)
        if be is None:
            return None
        w = be.wait_ge(carrier_sem, 0)
        ci = w.ins if hasattr(w, "ins") else w
        # strip from whatever block it was appended to
        for bb in nc.m.functions[0].blocks:
            il = list(bb.instructions)
            if any(x is ci for x in il):
                bb.instructions = [x for x in il if x is not ci]
                break
        return ci

    made = _split_waits(nc, mybir, mk_carrier)
    print(f"split_waits: carriers={made}", flush=True)
    return nc


def make_in_maps(inputs):
    kwin, shards = _host_shard(inputs["node_attrs"], inputs["vectors"],
                               inputs["senders"], inputs["receivers"])
    na = inputs["node_attrs"]
    in_maps = []
    for c in range(NC):
        m = _pack_core(kwin, *shards[c])
        m["asw"] = np.ascontiguousarray(
            na[c * NPC:(c + 1) * NPC].reshape(NW, WIN, F)
            .transpose(1, 0, 2)).astype(BF16)
        in_maps.append({k: np.ascontiguousarray(v) for k, v in m.items()})
    return kwin, in_maps


def kernel(**inputs):
    inputs = {k: np.asarray(v) for k, v in inputs.items()}
    kwin, in_maps = make_in_maps(inputs)
    nc = build_graph(kwin, _prep_weights(inputs))
    from concourse.bass_utils import run_bass_kernel_spmd
    res = run_bass_kernel_spmd(nc, in_maps, core_ids=list(range(NC)))
    out = np.zeros((128, RWIN), np.float64)
    for r in res.results:
        out += np.asarray(r["out"], np.float64)
    # node n = hi*128 + lo stored at [lo, hi]
    return np.ascontiguousarray(out.T.reshape(N, 1)).astype(np.float32)
